# revision 1
# baseline (speedup 1.0000x reference)
"""GAT layer on 8 Trainium2 NeuronCores (Bass/Tile), edge-parallel dst-sharded.

Self-contained: host preprocesses the graph (self-loops, dst-shard, bucket
sort, uniform bucket cap), the device program computes Wh/attention tables,
AllGathers the [Wh|el] table, then per 128-node bucket: indirect-DMA gathers
of table rows by src and er rows by dst, scores -> leaky-relu -> exp, and a
one-hot scatter matmul accumulated in PSUM, normalized and written out.
"""
import sys

for _p in ("/opt/trn_rl_repo",):
    if _p not in sys.path:
        sys.path.insert(0, _p)

import numpy as np
import ml_dtypes

import concourse.bass as bass
import concourse.tile as tile
from concourse import mybir
from concourse.bass import IndirectOffsetOnAxis
from concourse.bass_utils import run_bass_kernel_spmd

BF16 = ml_dtypes.bfloat16

N = 50000
E = 800000
IN = 256
H = 8
C = 32
HC = H * C            # 256
NC = 8
NPC = N // NC         # 6250 nodes per core
BUCKET = 128
NBUCK = (NPC + BUCKET - 1) // BUCKET   # 49
XT_PAD = NBUCK * 128                   # 6272
PAY = HC + H          # 264: table row [Wh(256) | el(8)]
P1COLS = HC + 2 * H   # 272: phase-1 matmul out [Wh | el | er]
NEG = 0.2
EPS = 1e-16
SC_BUCKETS = 2        # buckets per gather super-chunk

# walrus in this container caps sync waits per instruction at 1; hoist excess
# onto same-engine NoOps.
_waitfix_ctr = [0]


def _split_excess_waits(nc, max_waits=1):
    n_fixed = 0
    for fn in nc.m.functions:
        for bb in fn.blocks:
            insts = bb.instructions
            out = []
            for ins in insts:
                si = ins.sync_info
                waits = list(si.on_wait) if si is not None and si.on_wait else []
                if len(waits) > max_waits:
                    keep = waits[-max_waits:]
                    extra = waits[:-max_waits]
                    for i in range(0, len(extra), max_waits):
                        grp = extra[i:i + max_waits]
                        _waitfix_ctr[0] += 1
                        nop = mybir.InstNoOp(
                            name=f"I-waitfix-{_waitfix_ctr[0]}", ins=[], outs=[])
                        nop.engine = ins.engine
                        nop.sync_info = mybir.SyncInfo(on_wait=grp, on_update=[])
                        nc.register_instruction(nop)
                        out.append(nop)
                    si.on_wait = keep
                    n_fixed += 1
                out.append(ins)
            if len(out) != len(insts):
                bb.instructions = out
    return n_fixed


def _host_prep(x, edge_index, W, a_left, a_right):
    src = np.concatenate([edge_index[0], np.arange(N, dtype=np.int64)])
    dst = np.concatenate([edge_index[1], np.arange(N, dtype=np.int64)])
    src = src.astype(np.int64)
    dst = dst.astype(np.int64)

    # fold attention vectors through W:  [el|er] = x @ (W.T @ A)
    A = np.zeros((HC, 2 * H), np.float32)
    for h in range(H):
        A[h * C:(h + 1) * C, h] = a_left[h]
        A[h * C:(h + 1) * C, H + h] = a_right[h]
    B = (W.T.astype(np.float64) @ A.astype(np.float64)).astype(np.float32)
    wtb = np.concatenate([W.T.astype(np.float32), B], axis=1).astype(BF16)  # [256, 272]

    core = dst // NPC

    # uniform per-bucket cap across every core (SPMD: one program)
    gmax = 0
    for c in range(NC):
        m = core == c
        b = (dst[m] - c * NPC) // BUCKET
        cnt = np.bincount(b, minlength=NBUCK)
        gmax = max(gmax, int(cnt.max()))
    g_cap = ((gmax + 127) // 128) * 128
    nblkb = g_cap // 128          # blocks per bucket
    nblk = NBUCK * nblkb          # blocks per core

    goff = np.zeros((NC, 128, nblk), np.int32)
    eroff = np.zeros((NC, 128, nblk), np.int32)
    dloc = np.full((NC, 128, nblk), 200.0, BF16)
    xT = np.zeros((NC, IN, XT_PAD), BF16)

    for c in range(NC):
        m = core == c
        s_c = src[m]
        d_c = dst[m]
        dl = d_c - c * NPC
        b_c = dl // BUCKET
        order = np.lexsort((s_c, b_c))
        s_c, dl, b_c = s_c[order], dl[order], b_c[order]

        e_pad = NBUCK * g_cap
        src_p = np.zeros(e_pad, np.int64)
        dl_p = np.full(e_pad, 200.0, np.float32)
        erl_p = np.zeros(e_pad, np.int64)
        # bucket boundaries (b_c sorted)
        cnt = np.bincount(b_c, minlength=NBUCK)
        starts = np.concatenate([[0], np.cumsum(cnt)[:-1]])
        for b in range(NBUCK):
            g = cnt[b]
            o = b * g_cap
            sl = slice(starts[b], starts[b] + g)
            src_p[o:o + g] = s_c[sl]
            dl_p[o:o + g] = (dl[sl] - b * BUCKET).astype(np.float32)
            erl_p[o:o + g] = dl[sl]

        # edge slot k = (block j = k//128, partition p = k%128); SBUF array [p, j]
        goff[c] = src_p.reshape(nblk, 128).T
        eroff[c] = erl_p.reshape(nblk, 128).T
        dloc[c] = dl_p.reshape(nblk, 128).T.astype(BF16)

        xs = x[c * NPC:(c + 1) * NPC].astype(BF16)   # [6250, 256]
        xT[c, :, :NPC] = xs.T

    return wtb, goff, eroff, dloc, xT, g_cap


def _build_program(g_cap, debug=False):
    nblkb = g_cap // 128
    nblk = NBUCK * nblkb
    f32 = mybir.dt.float32
    bf16 = mybir.dt.bfloat16
    i32 = mybir.dt.int32

    nc = bass.Bass(trn_type="TRN2", num_devices=NC)
    xT_in = nc.declare_dram_parameter("xT", [IN, XT_PAD], bf16, isOutput=False)
    wtb_in = nc.declare_dram_parameter("wtb", [IN, P1COLS], bf16, isOutput=False)
    goff_in = nc.declare_dram_parameter("goff", [128, nblk], i32, isOutput=False)
    eroff_in = nc.declare_dram_parameter("eroff", [128, nblk], i32, isOutput=False)
    dloc_in = nc.declare_dram_parameter("dloc", [128, nblk], bf16, isOutput=False)
    out_ext = nc.declare_dram_parameter("out", [NPC, HC], f32, isOutput=True)
    if debug:
        dbg_tbl = nc.declare_dram_parameter("dbg_tbl", [NPC, PAY], bf16, isOutput=True)
        dbg_er = nc.declare_dram_parameter("dbg_er", [NPC, H], bf16, isOutput=True)
        dbg_g = nc.declare_dram_parameter("dbg_g", [128, PAY], bf16, isOutput=True)
        dbg_ere = nc.declare_dram_parameter("dbg_ere", [128, H], bf16, isOutput=True)
        dbg_w = nc.declare_dram_parameter("dbg_w", [128, H], bf16, isOutput=True)
        dbg_v = nc.declare_dram_parameter("dbg_v", [128, PAY], bf16, isOutput=True)
        dbg_ot = nc.declare_dram_parameter("dbg_ot", [128, 128], bf16, isOutput=True)
        dbg_ps = nc.declare_dram_parameter("dbg_ps", [128, PAY], f32, isOutput=True)
        dbg_ps1 = nc.declare_dram_parameter("dbg_ps1", [128, PAY], f32, isOutput=True)
        dbg_ps2 = nc.declare_dram_parameter("dbg_ps2", [128, PAY], f32, isOutput=True)

    tbl_loc = nc.dram_tensor("tbl_loc", [NPC, PAY], bf16)
    tbl_full = nc.dram_tensor("tbl_full", [N, PAY], bf16, addr_space="Shared")
    er_tbl = nc.dram_tensor("er_tbl", [NPC, H], bf16)

    with tile.TileContext(nc) as tc:
        # ---------------- phase 1: Wh / el / er ----------------
        with tc.tile_pool(name="p1w", bufs=1) as p1w, \
             tc.tile_pool(name="p1", bufs=3) as p1, \
             tc.tile_pool(name="ps1", bufs=2, space="PSUM") as ps1:
            xts = []
            wtbs = []
            for k in range(2):
                t = p1w.tile([128, XT_PAD], bf16, tag=f"xt{k}")
                nc.sync.dma_start(out=t[:], in_=xT_in[k * 128:(k + 1) * 128, :])
                xts.append(t)
                u = p1w.tile([128, P1COLS], bf16, tag=f"wtb{k}")
                nc.sync.dma_start(out=u[:], in_=wtb_in[k * 128:(k + 1) * 128, :])
                wtbs.append(u)
            for tn in range(NBUCK):
                ps = ps1.tile([128, P1COLS], f32)
                for k in range(2):
                    nc.tensor.matmul(
                        out=ps[:],
                        lhsT=xts[k][:, tn * 128:(tn + 1) * 128],
                        rhs=wtbs[k][:],
                        start=(k == 0), stop=(k == 1),
                    )
                sb = p1.tile([128, P1COLS], bf16)
                nc.vector.tensor_copy(out=sb[:], in_=ps[:])
                rows = min(128, NPC - tn * 128)
                nc.sync.dma_start(
                    out=tbl_loc[tn * 128:tn * 128 + rows, :], in_=sb[:rows, 0:PAY])
                nc.sync.dma_start(
                    out=er_tbl[tn * 128:tn * 128 + rows, :], in_=sb[:rows, PAY:P1COLS])

        # ---------------- all-gather the [Wh|el] table ----------------
        nc.gpsimd.collective_compute(
            "AllGather", mybir.AluOpType.bypass,
            replica_groups=[list(range(NC))],
            ins=[tbl_loc[:].opt()], outs=[tbl_full[:].opt()],
        )
        if debug:
            nc.sync.dma_start(out=dbg_tbl[:, :], in_=tbl_loc[:, :])
            nc.sync.dma_start(out=dbg_er[:, :], in_=er_tbl[:, :])

        # ---------------- phase 2: gather / score / scatter ----------------
        with tc.tile_pool(name="cst", bufs=1) as cst, \
             tc.tile_pool(name="gp", bufs=2) as gp, \
             tc.tile_pool(name="vp", bufs=24) as vp, \
             tc.tile_pool(name="otp", bufs=24) as otp, \
             tc.tile_pool(name="sp", bufs=2) as sp, \
             tc.tile_pool(name="np_", bufs=3) as np_, \
             tc.tile_pool(name="ps2", bufs=2, space="PSUM") as ps2p:

            iota_i = cst.tile([128, 128], i32)
            nc.gpsimd.iota(iota_i[:], pattern=[[1, 128]], base=0, channel_multiplier=0)
            iota_b = cst.tile([128, 128], bf16)
            nc.vector.tensor_copy(out=iota_b[:], in_=iota_i[:])

            goff_sb = cst.tile([128, nblk], i32)
            nc.sync.dma_start(out=goff_sb[:], in_=goff_in[:, :])
            eroff_sb = cst.tile([128, nblk], i32)
            nc.sync.dma_start(out=eroff_sb[:], in_=eroff_in[:, :])
            dloc_sb = cst.tile([128, nblk], bf16)
            nc.sync.dma_start(out=dloc_sb[:], in_=dloc_in[:, :])

            # HW indirect DMA consumes ONE offset per partition, streaming the
            # full per-partition output free-size contiguously (probe-verified)
            # -> one gather call per 128-edge block with [128, 1] offsets.
            er_e = cst.tile([128, nblk * H], bf16)
            er_e3 = er_e[:].rearrange("p (b h) -> p b h", h=H)
            for blk in range(nblk):
                nc.gpsimd.indirect_dma_start(
                    out=er_e3[:, blk, :], out_offset=None,
                    in_=er_tbl[:],
                    in_offset=IndirectOffsetOnAxis(
                        ap=eroff_sb[:, blk:blk + 1], axis=0),
                )

            n_sc = (NBUCK + SC_BUCKETS - 1) // SC_BUCKETS
            for sc in range(n_sc):
                b0 = sc * SC_BUCKETS
                nb_buckets = min(SC_BUCKETS, NBUCK - b0)
                nb = nb_buckets * nblkb              # blocks this super-chunk
                blk0 = b0 * nblkb

                G = gp.tile([128, nb * PAY], bf16)
                G3 = G[:].rearrange("p (b y) -> p b y", y=PAY)
                for blk in range(nb):
                    nc.gpsimd.indirect_dma_start(
                        out=G3[:, blk, :], out_offset=None,
                        in_=tbl_full[:],
                        in_offset=IndirectOffsetOnAxis(
                            ap=goff_sb[:, blk0 + blk:blk0 + blk + 1], axis=0),
                    )

                # scores: e = el + er ; leaky ; exp
                e_t = sp.tile([128, nb * H], f32, tag="e")
                e3 = e_t[:].rearrange("p (b h) -> p b h", h=H)
                nc.vector.tensor_tensor(
                    out=e3, in0=G3[:, :, HC:PAY],
                    in1=er_e3[:, blk0:blk0 + nb, :], op=mybir.AluOpType.add)
                es_t = sp.tile([128, nb * H], f32, tag="es")
                nc.vector.tensor_scalar_mul(es_t[:], e_t[:], NEG)
                nc.vector.tensor_tensor(
                    out=e_t[:], in0=e_t[:], in1=es_t[:], op=mybir.AluOpType.max)
                w_t = sp.tile([128, nb * H], bf16, tag="w")
                nc.scalar.activation(out=w_t[:], in_=e_t[:],
                                     func=mybir.ActivationFunctionType.Exp)
                w3 = w_t[:].rearrange("p (b h) -> p b h", h=H)

                # per-block V/OT tiles: matmul operands MUST be offset-0 APs —
                # a moving-operand free offset >= its inner count mis-lowers
                # (folds into the partition axis; verified on HW).
                i2 = iota_b[:]
                V_blks = []
                OT_blks = []
                for blk in range(nb):
                    Vb = vp.tile([128, PAY], bf16, tag="vblk")
                    G4b = G3[:, blk, 0:HC].rearrange("p (h c) -> p h c", c=C)
                    V4b = Vb[:, 0:HC].rearrange("p (h c) -> p h c", c=C)
                    w4b = w3[:, blk, :].to_broadcast([128, H, C])
                    nc.vector.tensor_tensor(out=V4b, in0=G4b, in1=w4b,
                                            op=mybir.AluOpType.mult)
                    nc.scalar.activation(out=Vb[:, HC:PAY], in_=w3[:, blk, :],
                                         func=mybir.ActivationFunctionType.Copy)
                    OTb = otp.tile([128, 128], bf16, tag="otblk")
                    db = dloc_sb[:, blk0 + blk].to_broadcast([128, 128])
                    nc.vector.tensor_tensor(out=OTb[:], in0=db, in1=i2,
                                            op=mybir.AluOpType.is_equal)
                    V_blks.append(Vb)
                    OT_blks.append(OTb)

                if debug and sc == 0:
                    nc.sync.dma_start(out=dbg_g[:, :], in_=G3[:, 0, :])
                    nc.sync.dma_start(out=dbg_ere[:, :], in_=er_e3[:, 0, :])
                    nc.sync.dma_start(out=dbg_w[:, :], in_=w3[:, 0, :])
                    nc.sync.dma_start(out=dbg_v[:, :], in_=V_blks[0][:, :])
                    nc.sync.dma_start(out=dbg_ot[:, :], in_=OT_blks[0][:, :])

                # scatter-accumulate per bucket, then normalize
                for bb in range(nb_buckets):
                    bucket = b0 + bb
                    ps = ps2p.tile([128, PAY], f32)
                    for j in range(nblkb):
                        blk = bb * nblkb + j
                        nc.tensor.matmul(
                            out=ps[:],
                            lhsT=OT_blks[blk][:],
                            rhs=V_blks[blk][:],
                            start=(j == 0), stop=(j == nblkb - 1),
                        )
                    if debug and bucket == 0:
                        ps_sb = np_.tile([128, PAY], f32, tag="psdump")
                        nc.vector.tensor_copy(out=ps_sb[:], in_=ps[:])
                        nc.sync.dma_start(out=dbg_ps[:, :], in_=ps_sb[:, :])
                    den = np_.tile([128, H], f32, tag="den")
                    nc.vector.tensor_scalar_add(den[:], ps[:, HC:PAY], EPS)
                    rec = np_.tile([128, H], f32, tag="rec")
                    nc.vector.reciprocal(rec[:], den[:])
                    ot = np_.tile([128, HC], f32, tag="ot")
                    ot3 = ot[:].rearrange("p (h c) -> p h c", c=C)
                    n3 = ps[:, 0:HC].rearrange("p (h c) -> p h c", c=C)
                    r3 = rec[:].to_broadcast([128, H, C])
                    nc.vector.tensor_tensor(out=ot3, in0=n3, in1=r3,
                                            op=mybir.AluOpType.mult)
                    rows = min(128, NPC - bucket * 128)
                    nc.sync.dma_start(
                        out=out_ext[bucket * 128:bucket * 128 + rows, :],
                        in_=ot[:rows, :])

    _split_excess_waits(nc)
    return nc


def kernel(**inputs):
    x = np.asarray(inputs["x"], np.float32)
    edge_index = np.asarray(inputs["edge_index"])
    W = np.asarray(inputs["W"], np.float32)
    a_left = np.asarray(inputs["a_left"], np.float32)
    a_right = np.asarray(inputs["a_right"], np.float32)

    wtb, goff, eroff, dloc, xT, g_cap = _host_prep(x, edge_index, W, a_left, a_right)
    nc = _build_program(g_cap)

    in_maps = []
    for c in range(NC):
        in_maps.append({
            "xT": np.ascontiguousarray(xT[c]),
            "wtb": wtb,
            "goff": np.ascontiguousarray(goff[c]),
            "eroff": np.ascontiguousarray(eroff[c]),
            "dloc": np.ascontiguousarray(dloc[c]),
        })

    res = run_bass_kernel_spmd(nc, in_maps, core_ids=list(range(NC)))
    out = np.concatenate([np.asarray(res.results[c]["out"]) for c in range(NC)], axis=0)
    return out.astype(np.float32)



# revision 13
# speedup vs baseline: 1.2827x; 1.2827x over previous
"""GAT layer on 8 Trainium2 NeuronCores (Bass/Tile), edge-parallel dst-sharded.

Self-contained. Host preprocesses the graph (self-loops, dst-shard, bucket
sort with a uniform bucket cap, A/B split of each bucket's edges by source
half for int16 gather indices). Device program:

  phase 1: per 128-node tile, matmul x @ [W.T | a-folded] producing rows
    [Wh1(264) | el(8) | er(8)] where Wh1 interleaves a constant 1.0 after
    each head's 32 channels ([Wh_h(32) | 1] x 8) so a single multiply by
    w=exp(leaky(e)) later yields both the weighted message and the softmax
    denominator column. Rows go to two local half-tables (stride 384 =
    dma_gather's 256B-multiple requirement) plus a local padded er table.
  AllGather x2: half-tables -> tblA/tblB on every core (the first AG fires
    mid-phase-1 and overlaps the rest; A-side gathers only wait on AG1).
  phase 2 per super-chunk of buckets: three dma_gather calls (table rows by
    src from tblA/tblB, er rows by dst), batched score ops
    (add -> leaky -> exp), one-op one-hot build (is_equal vs host iota),
    one-op V multiply, then per bucket capA+capB one-hot scatter matmuls
    accumulated in PSUM, normalized by the gathered denominator column and
    written out.

dma_gather is the key to speed here: one GpSimd (SWDGE) call per ~2.5k edges
instead of one indirect_dma_start per 128 edges (the descriptor-generation
fixed cost ~1us/call serializes on the Pool engine and dominated the
baseline at 1764 calls).
"""
import sys

for _p in ("/opt/trn_rl_repo",):
    if _p not in sys.path:
        sys.path.insert(0, _p)

import numpy as np
import ml_dtypes

import concourse.bass as bass
import concourse.tile as tile
from concourse import mybir, library_config
from concourse.bass_utils import run_bass_kernel_spmd
from concourse.ap_utils import ap_is_contiguous
from concourse.library_overlay import lower_extended_insts

BF16 = ml_dtypes.bfloat16

N = 50000
E = 800000
IN = 256
H = 8
C = 32
C1 = C + 1            # 33: [Wh_h(32) | 1]
HC = H * C            # 256
WH1 = H * C1          # 264
NC = 8
NPC = N // NC         # 6250 nodes per core
BUCKET = 128
NBUCK = (NPC + BUCKET - 1) // BUCKET   # 49
XT_PAD = NBUCK * 128                   # 6272
PAY = WH1 + H         # 272: gather payload [Wh1(264) | el(8)]
P1COLS = PAY + H      # 280: phase-1 matmul out [Wh1 | el | er]
TROW = 384            # table row stride (256B-multiple for dma_gather)
GELEM = 272           # gathered elements per row (272 if raw-IR works, else 384)
NEG = 0.2
EPS = 1e-16
SC_BUCKETS = 2        # buckets per gather super-chunk
NBUCK_A = 25          # buckets 0..24 -> rows 0..3199 (A half)
ROWS_A = NBUCK_A * BUCKET          # 3200
ROWS_B = NPC - ROWS_A              # 3050

_waitfix_ctr = [0]


def _split_excess_waits(nc, max_waits=1):
    # walrus in this container caps sync waits per instruction at 1; hoist
    # excess onto same-engine NoOps.
    n_fixed = 0
    for fn in nc.m.functions:
        for bb in fn.blocks:
            insts = bb.instructions
            out = []
            for ins in insts:
                si = ins.sync_info
                waits = list(si.on_wait) if si is not None and si.on_wait else []
                if len(waits) > max_waits:
                    keep = waits[-max_waits:]
                    extra = waits[:-max_waits]
                    for i in range(0, len(extra), max_waits):
                        grp = extra[i:i + max_waits]
                        _waitfix_ctr[0] += 1
                        nop = mybir.InstNoOp(
                            name=f"I-waitfix-{_waitfix_ctr[0]}", ins=[], outs=[])
                        nop.engine = ins.engine
                        nop.sync_info = mybir.SyncInfo(on_wait=grp, on_update=[])
                        nc.register_instruction(nop)
                        out.append(nop)
                    si.on_wait = keep
                    n_fixed += 1
                out.append(ins)
            if len(out) != len(insts):
                bb.instructions = out
    return n_fixed


_reg_cache = {}


def _move_reload_after_collectives(nc):
    """The tile scheduler floats the dependency-less library-reload pseudo to
    the top of the program; the collective trigger needs the default Q7
    library, so relocate the reload to just after the last collective."""
    from concourse import bass_isa
    for fn in nc.m.functions:
        for bb in fn.blocks:
            insts = bb.instructions
            reload_idx = [i for i, ins in enumerate(insts)
                          if isinstance(ins, bass_isa.InstPseudoReloadLibraryIndex)]
            coll_idx = [i for i, ins in enumerate(insts)
                        if isinstance(ins, mybir.InstCollectiveCompute)]
            if not reload_idx or not coll_idx:
                continue
            assert len(reload_idx) == 1
            r = reload_idx[0]
            last_c = max(coll_idx)
            if r > last_c:
                continue
            ins = insts.pop(r)
            insts.insert(last_c, ins)  # last_c shifted down by the pop
            bb.instructions = insts
    return nc


def _dma_gather_raw(eng, out_ap, in_ap, idxs_ap, num_idxs, elem_size, elem_step):
    """bass.dma_gather without the elem_size_bytes%256 assert (non-transpose,
    DRAM source). The row stride (elem_step bytes) must be a 256B multiple."""
    assert idxs_ap.dtype == mybir.dt.int16
    assert in_ap.dtype == out_ap.dtype
    assert ap_is_contiguous(out_ap.ap[1:])
    assert ap_is_contiguous(idxs_ap.ap[1:])
    assert in_ap.ap[0][0] == elem_step
    stride_bytes = elem_step * mybir.dt.size(in_ap.dtype)
    stride_bytes_256 = stride_bytes // 256
    assert stride_bytes_256 * 256 == stride_bytes and stride_bytes_256 < 256
    _in_ap = eng.lower_ap_dma(in_ap, for_custom_bir_dma=True)
    _idxs_ap = eng.lower_ap(idxs_ap)
    _out_ap = eng.lower_ap(out_ap)
    key = (id(eng.bass), num_idxs)
    if key not in _reg_cache:
        _reg_cache[key] = eng.to_reg(num_idxs)
    return eng.add_instruction(
        mybir.InstDMAGatherAnt(
            name=eng.bass.get_next_instruction_name(),
            ins=[*_in_ap, _idxs_ap, eng.lower_val_access(_reg_cache[key])],
            outs=[_out_ap],
            transpose=False,
            num_idxs=num_idxs,
            elem_size=elem_size,
            stride_bytes_256=stride_bytes_256,
            gen_mode=0,
            single_packet=False,
            queue_num=0,
            sbuf_tokens_per_rank=0,
            sbuf_free_dim_per_rank=0,
            sbuf_free_dim_pad_per_rank=0,
            sbuf_byte_offset=0,
        )
    )


def _wrap16(vals):
    """Edge-slot int16 index array -> dma_gather layout [128, n/16]
    (slot i at [i%16, i//16], replicated to all 16-partition groups)."""
    n = len(vals)
    assert n % 16 == 0
    w = np.asarray(vals, np.int16).reshape(n // 16, 16).T  # [16, n/16]
    return np.tile(w, (8, 1))


def _host_prep(x, edge_index, W, a_left, a_right):
    src = np.concatenate([edge_index[0], np.arange(N, dtype=np.int64)]).astype(np.int64)
    dst = np.concatenate([edge_index[1], np.arange(N, dtype=np.int64)]).astype(np.int64)

    # fold attention vectors through W:  [el|er] = x @ (W.T @ A)
    A = np.zeros((HC, 2 * H), np.float32)
    for h in range(H):
        A[h * C:(h + 1) * C, h] = a_left[h]
        A[h * C:(h + 1) * C, H + h] = a_right[h]
    B = (W.T.astype(np.float64) @ A.astype(np.float64)).astype(np.float32)
    # wtb columns: [0:264] interleaved [W.T head h | zero] x 8 (zero col gets
    # 1.0 on device), [264:272] el, [272:280] er.
    wtb = np.zeros((IN, P1COLS), np.float32)
    for h in range(H):
        wtb[:, h * C1:h * C1 + C] = W.T[:, h * C:(h + 1) * C]
    wtb[:, WH1:WH1 + H] = B[:, :H]
    wtb[:, WH1 + H:] = B[:, H:]
    wtb = wtb.astype(BF16)

    core = dst // NPC
    r_src = src % NPC
    c_src = src // NPC
    is_a = r_src < ROWS_A
    gidx = np.where(is_a, c_src * ROWS_A + r_src,
                    c_src * ROWS_B + (r_src - ROWS_A)).astype(np.int64)

    # per (core, bucket) A/B edge lists; two passes for uniform caps
    capA = capB = 0
    lists = {}
    for c in range(NC):
        m = core == c
        s_c, d_c, g_c, a_c = src[m], dst[m], gidx[m], is_a[m]
        dl = d_c - c * NPC
        b_c = dl // BUCKET
        order = np.lexsort((s_c, b_c))
        s_c, dl, b_c, g_c, a_c = (s_c[order], dl[order], b_c[order],
                                  g_c[order], a_c[order])
        cnt = np.bincount(b_c, minlength=NBUCK)
        starts = np.concatenate([[0], np.cumsum(cnt)[:-1]])
        for b in range(NBUCK):
            sl = slice(starts[b], starts[b] + cnt[b])
            aa = a_c[sl]
            lists[(c, b)] = (g_c[sl][aa], dl[sl][aa] - b * BUCKET, dl[sl][aa],
                             g_c[sl][~aa], dl[sl][~aa] - b * BUCKET, dl[sl][~aa])
            capA = max(capA, (int(aa.sum()) + 127) // 128)
            capB = max(capB, (int((~aa).sum()) + 127) // 128)

    nblkb = capA + capB
    nblk = NBUCK * nblkb
    nblkA = NBUCK * capA
    nblkB = NBUCK * capB
    n_sc = (NBUCK + SC_BUCKETS - 1) // SC_BUCKETS

    idxA = np.zeros((NC, 128, nblkA * 8), np.int16)
    idxB = np.zeros((NC, 128, nblkB * 8), np.int16)
    erloc = np.zeros((NC, 128, nblk * 8), np.int16)
    dloc_u = np.full((NC, 128, nblk), 200.0, BF16)
    xT = np.zeros((NC, IN, XT_PAD), BF16)

    for c in range(NC):
        iA = np.zeros(nblkA * 128, np.int64)
        iB = np.zeros(nblkB * 128, np.int64)
        dA = np.full(nblkA * 128, 200.0, np.float32)
        dB = np.full(nblkB * 128, 200.0, np.float32)
        eA = np.zeros(nblkA * 128, np.int64)
        eB = np.zeros(nblkB * 128, np.int64)
        for b in range(NBUCK):
            gA, dlA, erA, gB, dlB, erB = lists[(c, b)]
            oa = b * capA * 128
            ob = b * capB * 128
            iA[oa:oa + len(gA)] = gA
            dA[oa:oa + len(gA)] = dlA
            eA[oa:oa + len(gA)] = erA
            iB[ob:ob + len(gB)] = gB
            dB[ob:ob + len(gB)] = dlB
            eB[ob:ob + len(gB)] = erB
        idxA[c] = _wrap16(iA)
        idxB[c] = _wrap16(iB)
        # SC-ordered unified arrays: per SC [A-blocks | B-blocks]
        er_u = np.zeros(nblk * 128, np.int64)
        off = 0
        for sc in range(n_sc):
            b0 = sc * SC_BUCKETS
            bs = range(b0, min(b0 + SC_BUCKETS, NBUCK))
            for b in bs:
                blkcol = off // 128
                sl = slice(b * capA * 128, (b + 1) * capA * 128)
                er_u[off:off + capA * 128] = eA[sl]
                dloc_u[c, :, blkcol:blkcol + capA] = (
                    dA[sl].reshape(capA, 128).T.astype(BF16))
                off += capA * 128
            for b in bs:
                blkcol = off // 128
                sl = slice(b * capB * 128, (b + 1) * capB * 128)
                er_u[off:off + capB * 128] = eB[sl]
                dloc_u[c, :, blkcol:blkcol + capB] = (
                    dB[sl].reshape(capB, 128).T.astype(BF16))
                off += capB * 128
        erloc[c] = _wrap16(er_u)

        xs = x[c * NPC:(c + 1) * NPC].astype(BF16)   # [6250, 256]
        xT[c, :, :NPC] = xs.T

    iota = np.tile(np.arange(128, dtype=np.float32)[None, :], (128, 1)).astype(BF16)
    return wtb, idxA, idxB, erloc, dloc_u, xT, iota, capA, capB


def _build_program(capA, capB):
    nblkb = capA + capB
    nblk = NBUCK * nblkb
    nblkA = NBUCK * capA
    nblkB = NBUCK * capB
    f32 = mybir.dt.float32
    bf16 = mybir.dt.bfloat16
    i16 = mybir.dt.int16

    nc = bass.Bass(trn_type="TRN2", num_devices=NC)
    xT_in = nc.declare_dram_parameter("xT", [IN, XT_PAD], bf16, isOutput=False)
    wtb_in = nc.declare_dram_parameter("wtb", [IN, P1COLS], bf16, isOutput=False)
    idxA_in = nc.declare_dram_parameter("idxA", [128, nblkA * 8], i16, isOutput=False)
    idxB_in = nc.declare_dram_parameter("idxB", [128, nblkB * 8], i16, isOutput=False)
    erloc_in = nc.declare_dram_parameter("erloc", [128, nblk * 8], i16, isOutput=False)
    dloc_in = nc.declare_dram_parameter("dloc", [128, nblk], bf16, isOutput=False)
    iota_in = nc.declare_dram_parameter("iota", [128, 128], bf16, isOutput=False)
    out_ext = nc.declare_dram_parameter("out", [NPC, HC], f32, isOutput=True)

    tbl_locA = nc.dram_tensor("tbl_locA", [ROWS_A, TROW], bf16)
    tbl_locB = nc.dram_tensor("tbl_locB", [ROWS_B, TROW], bf16)
    tblA = nc.dram_tensor("tblA", [NC * ROWS_A, TROW], bf16, addr_space="Shared")
    tblB = nc.dram_tensor("tblB", [NC * ROWS_B, TROW], bf16, addr_space="Shared")
    er_pad = nc.dram_tensor("er_pad", [NPC, 128], bf16)

    with tile.TileContext(nc) as tc:
        with tc.tile_pool(name="cst", bufs=1) as cst:
            # ---------------- phase 1: Wh1 / el / er ----------------
            with tc.tile_pool(name="p1w", bufs=1) as p1w, \
                 tc.tile_pool(name="p1", bufs=3) as p1, \
                 tc.tile_pool(name="ps1", bufs=2, space="PSUM") as ps1:
                xts = []
                wtbs = []
                for k in range(2):
                    t = p1w.tile([128, XT_PAD], bf16, tag=f"xt{k}")
                    nc.sync.dma_start(out=t[:], in_=xT_in[k * 128:(k + 1) * 128, :])
                    xts.append(t)
                    u = p1w.tile([128, P1COLS], bf16, tag=f"wtb{k}")
                    nc.sync.dma_start(out=u[:], in_=wtb_in[k * 128:(k + 1) * 128, :])
                    wtbs.append(u)
                for tn in range(NBUCK):
                    ps = ps1.tile([128, P1COLS], f32)
                    for k in range(2):
                        nc.tensor.matmul(
                            out=ps[:],
                            lhsT=xts[k][:, tn * 128:(tn + 1) * 128],
                            rhs=wtbs[k][:],
                            start=(k == 0), stop=(k == 1),
                        )
                    sb = p1.tile([128, P1COLS], bf16)
                    nc.scalar.activation(out=sb[:], in_=ps[:],
                                         func=mybir.ActivationFunctionType.Copy)
                    sb4 = sb[:, 0:WH1].rearrange("p (h c) -> p h c", c=C1)
                    nc.vector.memset(sb4[:, :, C:C1], 1.0)
                    rows = min(128, NPC - tn * 128)
                    if tn < NBUCK_A:
                        trow = tbl_locA[tn * 128:tn * 128 + rows, 0:PAY]
                    else:
                        r0 = tn * 128 - ROWS_A
                        trow = tbl_locB[r0:r0 + rows, 0:PAY]
                    nc.sync.dma_start(out=trow, in_=sb[:rows, 0:PAY])
                    nc.sync.dma_start(
                        out=er_pad[tn * 128:tn * 128 + rows, 0:H],
                        in_=sb[:rows, PAY:P1COLS])

            # ---------------- all-gather both half tables ----------------
            nc.gpsimd.collective_compute(
                "AllGather", mybir.AluOpType.bypass,
                replica_groups=[list(range(NC))],
                ins=[tbl_locA[:].opt()], outs=[tblA[:].opt()],
            )
            nc.gpsimd.collective_compute(
                "AllGather", mybir.AluOpType.bypass,
                replica_groups=[list(range(NC))],
                ins=[tbl_locB[:].opt()], outs=[tblB[:].opt()],
            )
            # the collective trigger needs the default Q7 library; swap to
            # mlp (dma_gather) only after both AG triggers are on the queue
            nc.gpsimd.load_library(library_config.mlp)

            # ---------------- phase 2: gather / score / scatter ----------------
            with tc.tile_pool(name="gp", bufs=2) as gp, \
                 tc.tile_pool(name="erp", bufs=2) as erp, \
                 tc.tile_pool(name="vp", bufs=2) as vp, \
                 tc.tile_pool(name="otp", bufs=2) as otp, \
                 tc.tile_pool(name="sp", bufs=2) as sp, \
                 tc.tile_pool(name="np_", bufs=3) as np_, \
                 tc.tile_pool(name="ps2", bufs=4, space="PSUM") as ps2p:

                iota_b = cst.tile([128, 128], bf16)
                nc.sync.dma_start(out=iota_b[:], in_=iota_in[:, :])
                iota_m = iota_b[:].rearrange("p (b n) -> p b n", b=1)
                idxA_sb = cst.tile([128, nblkA * 8], i16)
                nc.sync.dma_start(out=idxA_sb[:], in_=idxA_in[:, :])
                idxB_sb = cst.tile([128, nblkB * 8], i16)
                nc.sync.dma_start(out=idxB_sb[:], in_=idxB_in[:, :])
                erloc_sb = cst.tile([128, nblk * 8], i16)
                nc.sync.dma_start(out=erloc_sb[:], in_=erloc_in[:, :])
                dloc_sb = cst.tile([128, nblk], bf16)
                nc.sync.dma_start(out=dloc_sb[:], in_=dloc_in[:, :])

                n_sc = (NBUCK + SC_BUCKETS - 1) // SC_BUCKETS
                off_u = 0   # unified block offset (SC order)
                for sc in range(n_sc):
                    b0 = sc * SC_BUCKETS
                    nbk = min(SC_BUCKETS, NBUCK - b0)
                    nbA = nbk * capA
                    nbB = nbk * capB
                    nb = nbA + nbB

                    G = gp.tile([128, nb * GELEM], bf16, tag="G")
                    G3 = G[:].rearrange("p (b y) -> p b y", y=GELEM)
                    a0 = b0 * capA * 8
                    _dma_gather_raw(
                        nc.gpsimd, G3[:, 0:nbA, :], tblA[:],
                        idxA_sb[:, a0:a0 + nbA * 8],
                        nbA * 128, GELEM, TROW)
                    bb0 = b0 * capB * 8
                    _dma_gather_raw(
                        nc.gpsimd, G3[:, nbA:nb, :], tblB[:],
                        idxB_sb[:, bb0:bb0 + nbB * 8],
                        nbB * 128, GELEM, TROW)

                    ER = erp.tile([128, nb * 128], bf16, tag="ER")
                    ER3 = ER[:].rearrange("p (b y) -> p b y", y=128)
                    _dma_gather_raw(
                        nc.gpsimd, ER3[:, :, :], er_pad[:],
                        erloc_sb[:, off_u * 8:(off_u + nb) * 8],
                        nb * 128, 128, 128)

                    # scores: e = el + er ; leaky ; exp
                    e_t = sp.tile([128, nb * H], f32, tag="e")
                    e3 = e_t[:].rearrange("p (b h) -> p b h", h=H)
                    nc.vector.tensor_tensor(
                        out=e3, in0=G3[:, :, WH1:PAY],
                        in1=ER3[:, :, 0:H], op=mybir.AluOpType.add)
                    es_t = sp.tile([128, nb * H], f32, tag="es")
                    nc.vector.tensor_scalar_mul(es_t[:], e_t[:], NEG)
                    nc.vector.tensor_tensor(
                        out=e_t[:], in0=e_t[:], in1=es_t[:],
                        op=mybir.AluOpType.max)
                    w_t = sp.tile([128, nb * H], bf16, tag="w")
                    nc.scalar.activation(out=w_t[:], in_=e_t[:],
                                         func=mybir.ActivationFunctionType.Exp)
                    w3 = w_t[:].rearrange("p (b h) -> p b h", h=H)

                    # one-op one-hot build for the whole super-chunk
                    OT = otp.tile([128, nb * 128], bf16, tag="OT")
                    OT3 = OT[:].rearrange("p (b n) -> p b n", n=128)
                    d3 = dloc_sb[:, off_u:off_u + nb].to_broadcast([128, nb, 128])
                    i3 = iota_m.to_broadcast([128, nb, 128])
                    nc.vector.tensor_tensor(out=OT3, in0=d3, in1=i3,
                                            op=mybir.AluOpType.is_equal)

                    # one-op V multiply
                    V = vp.tile([128, nb * WH1], bf16, tag="V")
                    V3 = V[:].rearrange("p (b y) -> p b y", y=WH1)
                    V4 = V[:].rearrange("p (b h c) -> p b h c", h=H, c=C1)
                    G4 = G3[:, :, 0:WH1].rearrange("p b (h c) -> p b h c", c=C1)
                    w4 = w3.to_broadcast([128, nb, H, C1])
                    nc.vector.tensor_tensor(out=V4, in0=G4, in1=w4,
                                            op=mybir.AluOpType.mult)

                    # scatter-accumulate per bucket, then normalize
                    for bb in range(nbk):
                        bucket = b0 + bb
                        blks = ([bb * capA + j for j in range(capA)] +
                                [nbA + bb * capB + j for j in range(capB)])
                        ps = ps2p.tile([128, WH1], f32)
                        for i, blk in enumerate(blks):
                            nc.tensor.matmul(
                                out=ps[:],
                                lhsT=OT3[:, blk, :],
                                rhs=V3[:, blk, :],
                                start=(i == 0), stop=(i == len(blks) - 1),
                            )
                        ps4 = ps[:].rearrange("p (h c) -> p h c", c=C1)
                        den = np_.tile([128, H], f32, tag="den")
                        nc.vector.tensor_scalar_add(den[:], ps4[:, :, C], EPS)
                        rec = np_.tile([128, H], f32, tag="rec")
                        nc.vector.reciprocal(rec[:], den[:])
                        ot = np_.tile([128, HC], f32, tag="ot")
                        ot3 = ot[:].rearrange("p (h c) -> p h c", c=C)
                        r3 = rec[:].to_broadcast([128, H, C])
                        nc.vector.tensor_tensor(out=ot3, in0=ps4[:, :, 0:C],
                                                in1=r3, op=mybir.AluOpType.mult)
                        rows = min(128, NPC - bucket * 128)
                        nc.sync.dma_start(
                            out=out_ext[bucket * 128:bucket * 128 + rows, :],
                            in_=ot[:rows, :])
                    off_u += nb

    _split_excess_waits(nc)
    lower_extended_insts(nc)
    return nc


def kernel(**inputs):
    x = np.asarray(inputs["x"], np.float32)
    edge_index = np.asarray(inputs["edge_index"])
    W = np.asarray(inputs["W"], np.float32)
    a_left = np.asarray(inputs["a_left"], np.float32)
    a_right = np.asarray(inputs["a_right"], np.float32)

    wtb, idxA, idxB, erloc, dloc_u, xT, iota, capA, capB = _host_prep(
        x, edge_index, W, a_left, a_right)
    nc = _build_program(capA, capB)

    in_maps = []
    for c in range(NC):
        in_maps.append({
            "xT": np.ascontiguousarray(xT[c]),
            "wtb": wtb,
            "idxA": np.ascontiguousarray(idxA[c]),
            "idxB": np.ascontiguousarray(idxB[c]),
            "erloc": np.ascontiguousarray(erloc[c]),
            "dloc": np.ascontiguousarray(dloc_u[c]),
            "iota": iota,
        })

    res = run_bass_kernel_spmd(nc, in_maps, core_ids=list(range(NC)))
    out = np.concatenate([np.asarray(res.results[c]["out"]) for c in range(NC)], axis=0)
    return out.astype(np.float32)


# revision 21
# speedup vs baseline: 1.7377x; 1.3547x over previous
"""GAT layer on 8 Trainium2 NeuronCores (Bass/Tile), edge-parallel dst-sharded.

Self-contained. Host preprocesses the graph (dst-shard, bucket sort with
uniform caps, A/B split of each bucket's edges by source half for int16
gather indices; self-loops are NOT materialized as edges). Device program:

  phase 1: per 128-node tile, matmul x @ [W.T | a-folded] producing rows
    [Wh1(264) | el(8) | er(8)]; Wh1 interleaves a constant 1.0 after each
    head's 32 channels so a single multiply by w=exp(leaky(e)) yields both
    the weighted message AND the softmax-denominator column. Rows go to two
    local half-tables (stride 384 for dma_gather's 256B-stride rule) and a
    compact local er table.
  AllGather x2: half tables -> tblA/tblB on every core (AG1 fires once the
    first 25 buckets are written and overlaps the rest of phase 1).
  er expansion on the TENSOR engine (no DMA gather): per bucket, a
    transposed one-hot OTT (built from host-shipped per-slot dst indices)
    times the bucket's er rows gives per-edge-slot er for every block -
    packed into one PSUM tile per bucket and copied out. Runs during the
    AllGather window.
  phase 2 per super-chunk: two dma_gather calls (A/B) fetch table rows by
    src; score ops (add on DVE, leaky+exp on the scalar engine); one-op
    one-hot build; per-block V multiplies; per bucket capA+capB one-hot
    scatter matmuls accumulated in PSUM, then a final identity-matmul adds
    the analytically-computed self-loop contribution, and the bucket is
    normalized by the gathered denominator column and written out.

The SWDGE (GpSimd) descriptor path only carries the unavoidable per-edge
table gather; everything index-like that is bucket-local (er by dst, the
self loops) rides the tensor engine instead.
"""
import sys

for _p in ("/opt/trn_rl_repo",):
    if _p not in sys.path:
        sys.path.insert(0, _p)

import numpy as np
import ml_dtypes

import concourse.bass as bass
import concourse.tile as tile
from concourse import mybir, library_config
from concourse.bass_utils import run_bass_kernel_spmd
from concourse.ap_utils import ap_is_contiguous
from concourse.library_overlay import lower_extended_insts

BF16 = ml_dtypes.bfloat16

N = 50000
E = 800000
IN = 256
H = 8
C = 32
C1 = C + 1            # 33: [Wh_h(32) | 1]
HC = H * C            # 256
WH1 = H * C1          # 264
NC = 8
NPC = N // NC         # 6250 nodes per core
BUCKET = 128
NBUCK = (NPC + BUCKET - 1) // BUCKET   # 49
XT_PAD = NBUCK * 128                   # 6272
PAY = WH1 + H         # 272: gather payload [Wh1(264) | el(8)]
P1COLS = PAY + H      # 280: phase-1 matmul out [Wh1 | el | er]
TROW = 384            # table row stride (256B multiple)
GELEM = 272           # gathered elements per row
NEG = 0.2
EPS = 1e-16
SC_BUCKETS = 2        # buckets per gather super-chunk
NBUCK_A = 25          # buckets 0..24 -> table half A
ROWS_A = NBUCK_A * BUCKET          # 3200
ROWS_B = NPC - ROWS_A              # 3050
USE_PREP_TRIGGER = False
USE_ACT_LRELU = False

_waitfix_ctr = [0]
_reg_cache = {}


def _split_excess_waits(nc, max_waits=1):
    # walrus in this container caps sync waits per instruction at 1; hoist
    # excess onto same-engine NoOps.
    n_fixed = 0
    for fn in nc.m.functions:
        for bb in fn.blocks:
            insts = bb.instructions
            out = []
            for ins in insts:
                si = ins.sync_info
                waits = list(si.on_wait) if si is not None and si.on_wait else []
                if len(waits) > max_waits:
                    keep = waits[-max_waits:]
                    extra = waits[:-max_waits]
                    for i in range(0, len(extra), max_waits):
                        grp = extra[i:i + max_waits]
                        _waitfix_ctr[0] += 1
                        nop = mybir.InstNoOp(
                            name=f"I-waitfix-{_waitfix_ctr[0]}", ins=[], outs=[])
                        nop.engine = ins.engine
                        nop.sync_info = mybir.SyncInfo(on_wait=grp, on_update=[])
                        nc.register_instruction(nop)
                        out.append(nop)
                    si.on_wait = keep
                    n_fixed += 1
                out.append(ins)
            if len(out) != len(insts):
                bb.instructions = out
    return n_fixed


def _move_reload_after_collectives(nc):
    """The tile scheduler floats the dependency-less library-reload pseudo to
    the top of the program; keep it after the last collective trigger."""
    from concourse import bass_isa
    for fn in nc.m.functions:
        for bb in fn.blocks:
            insts = bb.instructions
            reload_idx = [i for i, ins in enumerate(insts)
                          if isinstance(ins, bass_isa.InstPseudoReloadLibraryIndex)]
            coll_idx = [i for i, ins in enumerate(insts)
                        if isinstance(ins, mybir.InstCollectiveCompute)]
            if not reload_idx or not coll_idx:
                continue
            assert len(reload_idx) == 1
            r = reload_idx[0]
            last_c = max(coll_idx)
            if r > last_c:
                continue
            ins = insts.pop(r)
            insts.insert(last_c, ins)
            bb.instructions = insts
    return nc


def _dma_gather_raw(eng, out_ap, in_ap, idxs_ap, num_idxs, elem_size, elem_step,
                    sem=None):
    """bass.dma_gather without the elem_size_bytes%256 assert (non-transpose,
    DRAM source, 256B-multiple row stride). sem!=None -> prepare_only."""
    assert idxs_ap.dtype == mybir.dt.int16
    assert in_ap.dtype == out_ap.dtype
    assert ap_is_contiguous(out_ap.ap[1:])
    assert ap_is_contiguous(idxs_ap.ap[1:])
    assert in_ap.ap[0][0] == elem_step
    stride_bytes = elem_step * mybir.dt.size(in_ap.dtype)
    stride_bytes_256 = stride_bytes // 256
    assert stride_bytes_256 * 256 == stride_bytes and stride_bytes_256 < 256
    _in_ap = eng.lower_ap_dma(in_ap, for_custom_bir_dma=True)
    _idxs_ap = eng.lower_ap(idxs_ap)
    _out_ap = eng.lower_ap(out_ap)
    key = (id(eng.bass), num_idxs)
    if key not in _reg_cache:
        _reg_cache[key] = eng.to_reg(num_idxs)
    inst = eng.add_instruction(
        mybir.InstDMAGatherAnt(
            name=eng.bass.get_next_instruction_name(),
            ins=[*_in_ap, _idxs_ap, eng.lower_val_access(_reg_cache[key])],
            outs=[_out_ap],
            transpose=False,
            num_idxs=num_idxs,
            elem_size=elem_size,
            stride_bytes_256=stride_bytes_256,
            gen_mode=int(sem is not None),
            single_packet=False,
            queue_num=0,
            sbuf_tokens_per_rank=0,
            sbuf_free_dim_per_rank=0,
            sbuf_free_dim_pad_per_rank=0,
            sbuf_byte_offset=0,
        )
    )
    if sem is not None:
        inst.then_inc(sem, 16)
        return eng._track_prepare_only(inst, 0)
    return inst


def _wrap16(vals):
    """Edge-slot int16 index array -> dma_gather layout [128, n/16]."""
    n = len(vals)
    assert n % 16 == 0
    w = np.asarray(vals, np.int16).reshape(n // 16, 16).T
    return np.tile(w, (8, 1))


def _host_prep(x, edge_index, W, a_left, a_right):
    src = np.asarray(edge_index[0], np.int64)
    dst = np.asarray(edge_index[1], np.int64)

    # fold attention vectors through W:  [el|er] = x @ (W.T @ A)
    A = np.zeros((HC, 2 * H), np.float32)
    for h in range(H):
        A[h * C:(h + 1) * C, h] = a_left[h]
        A[h * C:(h + 1) * C, H + h] = a_right[h]
    B = (W.T.astype(np.float64) @ A.astype(np.float64)).astype(np.float32)
    wtb = np.zeros((IN, P1COLS), np.float32)
    for h in range(H):
        wtb[:, h * C1:h * C1 + C] = W.T[:, h * C:(h + 1) * C]
    wtb[:, WH1:WH1 + H] = B[:, :H]
    wtb[:, WH1 + H:] = B[:, H:]
    wtb = wtb.astype(BF16)

    core = dst // NPC
    r_src = src % NPC
    c_src = src // NPC
    is_a = r_src < ROWS_A
    gidx = np.where(is_a, c_src * ROWS_A + r_src,
                    c_src * ROWS_B + (r_src - ROWS_A)).astype(np.int64)

    capA = capB = 0
    lists = {}
    for c in range(NC):
        m = core == c
        s_c, d_c, g_c, a_c = src[m], dst[m], gidx[m], is_a[m]
        dl = d_c - c * NPC
        b_c = dl // BUCKET
        order = np.lexsort((s_c, b_c))
        s_c, dl, b_c, g_c, a_c = (s_c[order], dl[order], b_c[order],
                                  g_c[order], a_c[order])
        cnt = np.bincount(b_c, minlength=NBUCK)
        starts = np.concatenate([[0], np.cumsum(cnt)[:-1]])
        for b in range(NBUCK):
            sl = slice(starts[b], starts[b] + cnt[b])
            aa = a_c[sl]
            lists[(c, b)] = (g_c[sl][aa], dl[sl][aa] - b * BUCKET,
                             g_c[sl][~aa], dl[sl][~aa] - b * BUCKET)
            capA = max(capA, (int(aa.sum()) + 127) // 128)
            capB = max(capB, (int((~aa).sum()) + 127) // 128)

    nblkb = capA + capB
    nblk = NBUCK * nblkb
    nblkA = NBUCK * capA
    nblkB = NBUCK * capB
    n_sc = (NBUCK + SC_BUCKETS - 1) // SC_BUCKETS

    idxA = np.zeros((NC, 128, nblkA * 8), np.int16)
    idxB = np.zeros((NC, 128, nblkB * 8), np.int16)
    dloc_u = np.full((NC, 128, nblk), 200.0, BF16)
    dlocT = np.full((NC, 128, nblk * 128), -1, np.int8)
    xT = np.zeros((NC, IN, XT_PAD), BF16)

    for c in range(NC):
        iA = np.zeros(nblkA * 128, np.int64)
        iB = np.zeros(nblkB * 128, np.int64)
        dA = np.full((nblkA, 128), -1, np.int64)
        dB = np.full((nblkB, 128), -1, np.int64)
        for b in range(NBUCK):
            gA, dlA, gB, dlB = lists[(c, b)]
            oa = b * capA * 128
            ob = b * capB * 128
            iA[oa:oa + len(gA)] = gA
            iB[ob:ob + len(gB)] = gB
            fa = dA[b * capA:(b + 1) * capA].reshape(-1)
            fa[:len(dlA)] = dlA
            fb = dB[b * capB:(b + 1) * capB].reshape(-1)
            fb[:len(dlB)] = dlB
        idxA[c] = _wrap16(iA)
        idxB[c] = _wrap16(iB)
        # slot k of block j = (partition k%128); dA rows are flat slot runs
        dA = dA.reshape(nblkA, 128)
        dB = dB.reshape(nblkB, 128)
        # dloc_u: SC-major order [per SC: A-blocks | B-blocks], [128, nblk]
        off = 0
        for sc in range(n_sc):
            b0 = sc * SC_BUCKETS
            bs = range(b0, min(b0 + SC_BUCKETS, NBUCK))
            for b in bs:
                blk = dA[b * capA:(b + 1) * capA]     # [capA, 128]
                v = np.where(blk < 0, 200.0, blk).astype(np.float32)
                dloc_u[c, :, off:off + capA] = v.T.astype(BF16)
                off += capA
            for b in bs:
                blk = dB[b * capB:(b + 1) * capB]
                v = np.where(blk < 0, 200.0, blk).astype(np.float32)
                dloc_u[c, :, off:off + capB] = v.T.astype(BF16)
                off += capB
        # dlocT: BUCKET-major order [per bucket: A-blocks | B-blocks],
        # transposed and replicated: [128(any), (b*nblkb + j)*128 + p]
        for b in range(NBUCK):
            base = b * nblkb * 128
            rows = np.concatenate(
                [dA[b * capA:(b + 1) * capA], dB[b * capB:(b + 1) * capB]],
                axis=0)                                # [nblkb, 128]
            dlocT[c, :, base:base + nblkb * 128] = np.broadcast_to(
                rows.reshape(-1).astype(np.int8), (128, nblkb * 128))

        xs = x[c * NPC:(c + 1) * NPC].astype(BF16)
        xT[c, :, :NPC] = xs.T

    iota = np.tile(np.arange(128, dtype=np.float32)[None, :], (128, 1)).astype(BF16)
    iotaP = np.arange(128, dtype=np.int8).reshape(128, 1)
    iotaPb = np.arange(128, dtype=np.float32).reshape(128, 1).astype(BF16)
    return (wtb, idxA, idxB, dloc_u, dlocT, xT, iota, iotaP, iotaPb,
            capA, capB)


def _build_program(capA, capB):
    nblkb = capA + capB
    nblk = NBUCK * nblkb
    nblkA = NBUCK * capA
    nblkB = NBUCK * capB
    f32 = mybir.dt.float32
    bf16 = mybir.dt.bfloat16
    i16 = mybir.dt.int16
    i8 = mybir.dt.int8

    nc = bass.Bass(trn_type="TRN2", num_devices=NC)
    xT_in = nc.declare_dram_parameter("xT", [IN, XT_PAD], bf16, isOutput=False)
    wtb_in = nc.declare_dram_parameter("wtb", [IN, P1COLS], bf16, isOutput=False)
    idxA_in = nc.declare_dram_parameter("idxA", [128, nblkA * 8], i16, isOutput=False)
    idxB_in = nc.declare_dram_parameter("idxB", [128, nblkB * 8], i16, isOutput=False)
    dloc_in = nc.declare_dram_parameter("dloc", [128, nblk], bf16, isOutput=False)
    dlocT_in = nc.declare_dram_parameter("dlocT", [128, nblk * 128], i8, isOutput=False)
    iota_in = nc.declare_dram_parameter("iota", [128, 128], bf16, isOutput=False)
    iotaP_in = nc.declare_dram_parameter("iotaP", [128, 1], i8, isOutput=False)
    iotaPb_in = nc.declare_dram_parameter("iotaPb", [128, 1], bf16, isOutput=False)
    out_ext = nc.declare_dram_parameter("out", [NPC, HC], f32, isOutput=True)

    tbl_locA = nc.dram_tensor("tbl_locA", [ROWS_A, TROW], bf16)
    tbl_locB = nc.dram_tensor("tbl_locB", [ROWS_B, TROW], bf16)
    tblA = nc.dram_tensor("tblA", [NC * ROWS_A, TROW], bf16, addr_space="Shared")
    tblB = nc.dram_tensor("tblB", [NC * ROWS_B, TROW], bf16, addr_space="Shared")
    er_tbl = nc.dram_tensor("er_tbl", [NBUCK * 128, H], bf16)

    with tile.TileContext(nc) as tc:
        with tc.tile_pool(name="cst", bufs=1) as cst:
            # ---------------- phase 1: Wh1 / el / er ----------------
            with tc.tile_pool(name="p1w", bufs=1) as p1w, \
                 tc.tile_pool(name="p1", bufs=3) as p1, \
                 tc.tile_pool(name="ps1", bufs=2, space="PSUM") as ps1:
                xts = []
                wtbs = []
                for k in range(2):
                    t = p1w.tile([128, XT_PAD], bf16, tag=f"xt{k}")
                    nc.sync.dma_start(out=t[:], in_=xT_in[k * 128:(k + 1) * 128, :])
                    xts.append(t)
                    u = p1w.tile([128, P1COLS], bf16, tag=f"wtb{k}")
                    nc.sync.dma_start(out=u[:], in_=wtb_in[k * 128:(k + 1) * 128, :])
                    wtbs.append(u)
                for tn in range(NBUCK):
                    ps = ps1.tile([128, P1COLS], f32)
                    for k in range(2):
                        nc.tensor.matmul(
                            out=ps[:],
                            lhsT=xts[k][:, tn * 128:(tn + 1) * 128],
                            rhs=wtbs[k][:],
                            start=(k == 0), stop=(k == 1),
                        )
                    sb = p1.tile([128, P1COLS], bf16)
                    nc.scalar.activation(out=sb[:], in_=ps[:],
                                         func=mybir.ActivationFunctionType.Copy)
                    sb4 = sb[:, 0:WH1].rearrange("p (h c) -> p h c", c=C1)
                    nc.vector.memset(sb4[:, :, C:C1], 1.0)
                    rows = min(128, NPC - tn * 128)
                    if tn < NBUCK_A:
                        trow = tbl_locA[tn * 128:tn * 128 + rows, 0:PAY]
                    else:
                        r0 = tn * 128 - ROWS_A
                        trow = tbl_locB[r0:r0 + rows, 0:PAY]
                    nc.sync.dma_start(out=trow, in_=sb[:rows, 0:PAY])
                    nc.sync.dma_start(
                        out=er_tbl[tn * 128:tn * 128 + rows, :],
                        in_=sb[:rows, PAY:P1COLS])

            # ---------------- all-gather both half tables ----------------
            nc.gpsimd.collective_compute(
                "AllGather", mybir.AluOpType.bypass,
                replica_groups=[list(range(NC))],
                ins=[tbl_locA[:].opt()], outs=[tblA[:].opt()],
            )
            nc.gpsimd.collective_compute(
                "AllGather", mybir.AluOpType.bypass,
                replica_groups=[list(range(NC))],
                ins=[tbl_locB[:].opt()], outs=[tblB[:].opt()],
            )
            nc.gpsimd.load_library(library_config.mlp)

            iota_b = cst.tile([128, 128], bf16)
            nc.sync.dma_start(out=iota_b[:], in_=iota_in[:, :])
            iota_m = iota_b[:].rearrange("p (b n) -> p b n", b=1)
            iotaP_sb = cst.tile([128, 1], i8)
            nc.sync.dma_start(out=iotaP_sb[:], in_=iotaP_in[:, :])
            iotaP_m = iotaP_sb[:].rearrange("p (b n) -> p b n", b=1)
            iotaPb_sb = cst.tile([128, 1], bf16)
            nc.sync.dma_start(out=iotaPb_sb[:], in_=iotaPb_in[:, :])
            ident = cst.tile([128, 128], bf16)
            nc.vector.tensor_tensor(
                out=ident[:], in0=iotaPb_sb[:].to_broadcast([128, 128]),
                in1=iota_b[:], op=mybir.AluOpType.is_equal)

            idxA_sb = cst.tile([128, nblkA * 8], i16)
            nc.sync.dma_start(out=idxA_sb[:], in_=idxA_in[:, :])
            idxB_sb = cst.tile([128, nblkB * 8], i16)
            nc.sync.dma_start(out=idxB_sb[:], in_=idxB_in[:, :])
            dloc_sb = cst.tile([128, nblk], bf16)
            nc.sync.dma_start(out=dloc_sb[:], in_=dloc_in[:, :])
            # er for all buckets, bucket-partition layout: [p, b, h]
            er_full = cst.tile([128, NBUCK * H], bf16)
            er_full3 = er_full[:].rearrange("p (b h) -> p b h", h=H)
            er_in3 = er_tbl[:(NBUCK - 1) * 128].rearrange("(b p) h -> p b h", p=128)
            nc.sync.dma_start(out=er_full3[:, 0:NBUCK - 1, :], in_=er_in3[:, :, :])
            # last bucket is partial (106 rows): zero first so the OTT matmul
            # can't pick up non-finite garbage from the dead partitions
            nc.vector.memset(er_full3[:, NBUCK - 1, :], 0.0)
            lastr = NPC - (NBUCK - 1) * 128
            nc.sync.dma_start(
                out=er_full3[0:lastr, NBUCK - 1, :],
                in_=er_tbl[(NBUCK - 1) * 128:NPC, :])

            # ---------------- er expansion on PE (overlaps AllGather) ------
            er_e = cst.tile([128, nblk * 8], bf16)   # SC-major slot order
            with tc.tile_pool(name="otq", bufs=2) as otq, \
                 tc.tile_pool(name="dtq", bufs=2) as dtq, \
                 tc.tile_pool(name="pse", bufs=2, space="PSUM") as pse:
                n_sc = (NBUCK + SC_BUCKETS - 1) // SC_BUCKETS
                for b in range(NBUCK):
                    dT = dtq.tile([128, nblkb * 128], i8, tag="dT")
                    nc.sync.dma_start(
                        out=dT[:],
                        in_=dlocT_in[:, b * nblkb * 128:(b + 1) * nblkb * 128])
                    dT3 = dT[:].rearrange("p (j n) -> p j n", n=128)
                    OTT = otq.tile([128, nblkb * 128], bf16, tag="OTT")
                    OTT3 = OTT[:].rearrange("p (j n) -> p j n", n=128)
                    nc.vector.tensor_tensor(
                        out=OTT3, in0=iotaP_m.to_broadcast([128, nblkb, 128]),
                        in1=dT3, op=mybir.AluOpType.is_equal)
                    pe = pse.tile([128, nblkb * H], f32)
                    pe3 = pe[:].rearrange("p (j h) -> p j h", h=H)
                    for j in range(nblkb):
                        nc.tensor.matmul(
                            out=pe3[:, j, :], lhsT=OTT3[:, j, :],
                            rhs=er_full3[:, b, :], start=True, stop=True)
                    # copy into er_e at SC-major positions (A-run | B-run)
                    sc = b // SC_BUCKETS
                    b0 = sc * SC_BUCKETS
                    nbk = min(SC_BUCKETS, NBUCK - b0)
                    off_sc = b0 * nblkb
                    offA = (off_sc + (b - b0) * capA) * 8
                    offB = (off_sc + nbk * capA + (b - b0) * capB) * 8
                    nc.vector.tensor_copy(
                        out=er_e[:, offA:offA + capA * 8],
                        in_=pe[:, 0:capA * 8])
                    nc.vector.tensor_copy(
                        out=er_e[:, offB:offB + capB * 8],
                        in_=pe[:, capA * 8:nblkb * 8])

            # ---------------- phase 2: gather / score / scatter ------------
            with tc.tile_pool(name="gp", bufs=2) as gp, \
                 tc.tile_pool(name="vp", bufs=2) as vp, \
                 tc.tile_pool(name="otp", bufs=2) as otp, \
                 tc.tile_pool(name="sp", bufs=2) as sp, \
                 tc.tile_pool(name="sf", bufs=3) as sf, \
                 tc.tile_pool(name="np_", bufs=3) as np_, \
                 tc.tile_pool(name="ps2", bufs=4, space="PSUM") as ps2p:

                gsem = nc.alloc_semaphore("gsem") if USE_PREP_TRIGGER else None
                n_sc = (NBUCK + SC_BUCKETS - 1) // SC_BUCKETS
                off_u = 0
                for sc in range(n_sc):
                    b0 = sc * SC_BUCKETS
                    nbk = min(SC_BUCKETS, NBUCK - b0)
                    nbA = nbk * capA
                    nbB = nbk * capB
                    nb = nbA + nbB

                    G = gp.tile([128, nb * GELEM], bf16, tag="G")
                    G3 = G[:].rearrange("p (b y) -> p b y", y=GELEM)
                    a0 = b0 * capA * 8
                    bb0 = b0 * capB * 8
                    if USE_PREP_TRIGGER:
                        _dma_gather_raw(
                            nc.gpsimd, G3[:, 0:nbA, :], tblA[:],
                            idxA_sb[:, a0:a0 + nbA * 8], nbA * 128,
                            GELEM, TROW, sem=gsem)
                        nc.gpsimd.trigger_dma(count=None)
                        _dma_gather_raw(
                            nc.gpsimd, G3[:, nbA:nb, :], tblB[:],
                            idxB_sb[:, bb0:bb0 + nbB * 8], nbB * 128,
                            GELEM, TROW, sem=gsem)
                        nc.gpsimd.trigger_dma(count=None)
                    else:
                        _dma_gather_raw(
                            nc.gpsimd, G3[:, 0:nbA, :], tblA[:],
                            idxA_sb[:, a0:a0 + nbA * 8], nbA * 128,
                            GELEM, TROW)
                        _dma_gather_raw(
                            nc.gpsimd, G3[:, nbA:nb, :], tblB[:],
                            idxB_sb[:, bb0:bb0 + nbB * 8], nbB * 128,
                            GELEM, TROW)

                    # scores: e = el + er ; leaky ; exp
                    e_t = sp.tile([128, nb * H], f32, tag="e")
                    e3 = e_t[:].rearrange("p (b h) -> p b h", h=H)
                    er_sc3 = er_e[:, off_u * 8:(off_u + nb) * 8].rearrange(
                        "p (b h) -> p b h", h=H)
                    nc.vector.tensor_tensor(
                        out=e3, in0=G3[:, :, WH1:PAY], in1=er_sc3,
                        op=mybir.AluOpType.add)
                    w_t = sp.tile([128, nb * H], bf16, tag="w")
                    if USE_ACT_LRELU:
                        el_t = sp.tile([128, nb * H], f32, tag="el")
                        nc.scalar.activation(
                            out=el_t[:], in_=e_t[:],
                            func=mybir.ActivationFunctionType.Lrelu, alpha=NEG)
                        nc.scalar.activation(
                            out=w_t[:], in_=el_t[:],
                            func=mybir.ActivationFunctionType.Exp)
                    else:
                        es_t = sp.tile([128, nb * H], f32, tag="es")
                        nc.vector.tensor_scalar_mul(es_t[:], e_t[:], NEG)
                        nc.vector.tensor_tensor(
                            out=e_t[:], in0=e_t[:], in1=es_t[:],
                            op=mybir.AluOpType.max)
                        nc.scalar.activation(
                            out=w_t[:], in_=e_t[:],
                            func=mybir.ActivationFunctionType.Exp)
                    w3 = w_t[:].rearrange("p (b h) -> p b h", h=H)

                    # one-op one-hot build for the whole super-chunk
                    OT = otp.tile([128, nb * 128], bf16, tag="OT")
                    OT3 = OT[:].rearrange("p (b n) -> p b n", n=128)
                    d3 = dloc_sb[:, off_u:off_u + nb].to_broadcast([128, nb, 128])
                    i3 = iota_m.to_broadcast([128, nb, 128])
                    nc.vector.tensor_tensor(out=OT3, in0=d3, in1=i3,
                                            op=mybir.AluOpType.is_equal)

                    # per-block V multiplies
                    V = vp.tile([128, nb * WH1], bf16, tag="V")
                    V3 = V[:].rearrange("p (b y) -> p b y", y=WH1)
                    for blk in range(nb):
                        V4b = V3[:, blk, :].rearrange("p (h c) -> p h c", c=C1)
                        G4b = G3[:, blk, 0:WH1].rearrange("p (h c) -> p h c", c=C1)
                        w4b = w3[:, blk, :].to_broadcast([128, H, C1])
                        nc.vector.tensor_tensor(out=V4b, in0=G4b, in1=w4b,
                                                op=mybir.AluOpType.mult)

                    # per bucket: scatter matmuls + self-loop fold + normalize
                    for bb in range(nbk):
                        bucket = b0 + bb
                        # self-loop contribution
                        ts = sf.tile([128, PAY], bf16, tag="ts")
                        if bucket < NBUCK_A:
                            tsrc = tbl_locA[bucket * 128:bucket * 128 + 128, 0:PAY]
                        else:
                            r0 = bucket * 128 - ROWS_A
                            rows = min(128, ROWS_B - r0)
                            tsrc = tbl_locB[r0:r0 + rows, 0:PAY]
                        nc.sync.dma_start(out=ts[:tsrc.shape[0], :], in_=tsrc)
                        es_s = sf.tile([128, H], f32, tag="es_s")
                        nc.vector.tensor_tensor(
                            out=es_s[:], in0=ts[:, WH1:PAY],
                            in1=er_full3[:, bucket, :], op=mybir.AluOpType.add)
                        ws_s = sf.tile([128, H], bf16, tag="ws_s")
                        if USE_ACT_LRELU:
                            nc.scalar.activation(
                                out=es_s[:], in_=es_s[:],
                                func=mybir.ActivationFunctionType.Lrelu, alpha=NEG)
                        else:
                            es_s2 = sf.tile([128, H], f32, tag="es_s2")
                            nc.vector.tensor_scalar_mul(es_s2[:], es_s[:], NEG)
                            nc.vector.tensor_tensor(
                                out=es_s[:], in0=es_s[:], in1=es_s2[:],
                                op=mybir.AluOpType.max)
                        nc.scalar.activation(
                            out=ws_s[:], in_=es_s[:],
                            func=mybir.ActivationFunctionType.Exp)
                        vs = sf.tile([128, WH1], bf16, tag="vs")
                        vs4 = vs[:].rearrange("p (h c) -> p h c", c=C1)
                        ts4 = ts[:, 0:WH1].rearrange("p (h c) -> p h c", c=C1)
                        nc.vector.tensor_tensor(
                            out=vs4, in0=ts4,
                            in1=ws_s[:].to_broadcast([128, H, C1]),
                            op=mybir.AluOpType.mult)

                        blks = ([bb * capA + j for j in range(capA)] +
                                [nbA + bb * capB + j for j in range(capB)])
                        ps = ps2p.tile([128, WH1], f32)
                        for i, blk in enumerate(blks):
                            nc.tensor.matmul(
                                out=ps[:], lhsT=OT3[:, blk, :], rhs=V3[:, blk, :],
                                start=(i == 0), stop=False)
                        nc.tensor.matmul(
                            out=ps[:], lhsT=ident[:], rhs=vs[:],
                            start=False, stop=True)

                        ps4 = ps[:].rearrange("p (h c) -> p h c", c=C1)
                        den = np_.tile([128, H], f32, tag="den")
                        nc.vector.tensor_scalar_add(den[:], ps4[:, :, C], EPS)
                        rec = np_.tile([128, H], f32, tag="rec")
                        nc.vector.reciprocal(rec[:], den[:])
                        ot = np_.tile([128, HC], f32, tag="ot")
                        ot3 = ot[:].rearrange("p (h c) -> p h c", c=C)
                        r3 = rec[:].to_broadcast([128, H, C])
                        nc.vector.tensor_tensor(out=ot3, in0=ps4[:, :, 0:C],
                                                in1=r3, op=mybir.AluOpType.mult)
                        rows = min(128, NPC - bucket * 128)
                        nc.sync.dma_start(
                            out=out_ext[bucket * 128:bucket * 128 + rows, :],
                            in_=ot[:rows, :])
                    off_u += nb

    _split_excess_waits(nc)
    _move_reload_after_collectives(nc)
    lower_extended_insts(nc)
    return nc


def kernel(**inputs):
    x = np.asarray(inputs["x"], np.float32)
    edge_index = np.asarray(inputs["edge_index"])
    W = np.asarray(inputs["W"], np.float32)
    a_left = np.asarray(inputs["a_left"], np.float32)
    a_right = np.asarray(inputs["a_right"], np.float32)

    (wtb, idxA, idxB, dloc_u, dlocT, xT, iota, iotaP, iotaPb,
     capA, capB) = _host_prep(x, edge_index, W, a_left, a_right)
    nc = _build_program(capA, capB)

    in_maps = []
    for c in range(NC):
        in_maps.append({
            "xT": np.ascontiguousarray(xT[c]),
            "wtb": wtb,
            "idxA": np.ascontiguousarray(idxA[c]),
            "idxB": np.ascontiguousarray(idxB[c]),
            "dloc": np.ascontiguousarray(dloc_u[c]),
            "dlocT": np.ascontiguousarray(dlocT[c]),
            "iota": iota,
            "iotaP": iotaP,
            "iotaPb": iotaPb,
        })

    res = run_bass_kernel_spmd(nc, in_maps, core_ids=list(range(NC)))
    out = np.concatenate([np.asarray(res.results[c]["out"]) for c in range(NC)], axis=0)
    return out.astype(np.float32)


# revision 23
# speedup vs baseline: 2.1205x; 1.2203x over previous
"""GAT layer on 8 Trainium2 NeuronCores (Bass/Tile), edge-parallel dst-sharded.

Self-contained. Host preprocesses the graph (dst-shard, bucket sort with
uniform caps, A/B split of each bucket's edges by source half for int16
gather indices; self-loops are NOT materialized as edges). Device program:

  phase 1: per 128-node tile, matmul x @ [W.T | a-folded] producing rows
    [Wh1(264) | el(8) | er(8)]; Wh1 interleaves a constant 1.0 after each
    head's 32 channels so a single multiply by w=exp(leaky(e)) yields both
    the weighted message AND the softmax-denominator column. Rows go to two
    local half-tables (stride 384 for dma_gather's 256B-stride rule) and a
    compact local er table.
  AllGather x2: half tables -> tblA/tblB on every core (AG1 fires once the
    first 25 buckets are written and overlaps the rest of phase 1).
  er expansion on the TENSOR engine (no DMA gather): per bucket, a
    transposed one-hot OTT (built from host-shipped per-slot dst indices)
    times the bucket's er rows gives per-edge-slot er for every block -
    packed into one PSUM tile per bucket and copied out. Runs during the
    AllGather window.
  phase 2 per super-chunk: two dma_gather calls (A/B) fetch table rows by
    src; score ops (add on DVE, leaky+exp on the scalar engine); one-op
    one-hot build; per-block V multiplies; per bucket capA+capB one-hot
    scatter matmuls accumulated in PSUM, then a final identity-matmul adds
    the analytically-computed self-loop contribution, and the bucket is
    normalized by the gathered denominator column and written out.

The SWDGE (GpSimd) descriptor path only carries the unavoidable per-edge
table gather; everything index-like that is bucket-local (er by dst, the
self loops) rides the tensor engine instead.
"""
import sys

for _p in ("/opt/trn_rl_repo",):
    if _p not in sys.path:
        sys.path.insert(0, _p)

import numpy as np
import ml_dtypes

import concourse.bass as bass
import concourse.tile as tile
from concourse import mybir, library_config
from concourse.bass_utils import run_bass_kernel_spmd
from concourse.ap_utils import ap_is_contiguous
from concourse.library_overlay import lower_extended_insts

BF16 = ml_dtypes.bfloat16

N = 50000
E = 800000
IN = 256
H = 8
C = 32
C1 = C + 1            # 33: [Wh_h(32) | 1]
HC = H * C            # 256
WH1 = H * C1          # 264
NC = 8
NPC = N // NC         # 6250 nodes per core
BUCKET = 128
NBUCK = (NPC + BUCKET - 1) // BUCKET   # 49
XT_PAD = NBUCK * 128                   # 6272
PAY = WH1 + H         # 272: gather payload [Wh1(264) | el(8)]
P1COLS = PAY + H      # 280: phase-1 matmul out [Wh1 | el | er]
TROW = 384            # table row stride (256B multiple)
GELEM = 272           # gathered elements per row
NEG = 0.2
EPS = 1e-16
SC_BUCKETS = 2        # buckets per gather super-chunk
NBUCK_A = 25          # buckets 0..24 -> table half A
ROWS_A = NBUCK_A * BUCKET          # 3200
ROWS_B = NPC - ROWS_A              # 3050
USE_PREP_TRIGGER = False
USE_ACT_LRELU = False

_waitfix_ctr = [0]
_reg_cache = {}


def _split_excess_waits(nc, max_waits=1):
    # walrus in this container caps sync waits per instruction at 1; hoist
    # excess onto same-engine NoOps.
    n_fixed = 0
    for fn in nc.m.functions:
        for bb in fn.blocks:
            insts = bb.instructions
            out = []
            for ins in insts:
                si = ins.sync_info
                waits = list(si.on_wait) if si is not None and si.on_wait else []
                if len(waits) > max_waits:
                    keep = waits[-max_waits:]
                    extra = waits[:-max_waits]
                    for i in range(0, len(extra), max_waits):
                        grp = extra[i:i + max_waits]
                        _waitfix_ctr[0] += 1
                        nop = mybir.InstNoOp(
                            name=f"I-waitfix-{_waitfix_ctr[0]}", ins=[], outs=[])
                        nop.engine = ins.engine
                        nop.sync_info = mybir.SyncInfo(on_wait=grp, on_update=[])
                        nc.register_instruction(nop)
                        out.append(nop)
                    si.on_wait = keep
                    n_fixed += 1
                out.append(ins)
            if len(out) != len(insts):
                bb.instructions = out
    return n_fixed


def _move_reload_after_collectives(nc):
    """The tile scheduler floats the dependency-less library-reload pseudo to
    the top of the program; keep it after the last collective trigger."""
    from concourse import bass_isa
    for fn in nc.m.functions:
        for bb in fn.blocks:
            insts = bb.instructions
            reload_idx = [i for i, ins in enumerate(insts)
                          if isinstance(ins, bass_isa.InstPseudoReloadLibraryIndex)]
            coll_idx = [i for i, ins in enumerate(insts)
                        if isinstance(ins, mybir.InstCollectiveCompute)]
            if not reload_idx or not coll_idx:
                continue
            assert len(reload_idx) == 1
            r = reload_idx[0]
            last_c = max(coll_idx)
            if r > last_c:
                continue
            ins = insts.pop(r)
            insts.insert(last_c, ins)
            bb.instructions = insts
    return nc


def _dma_gather_raw(eng, out_ap, in_ap, idxs_ap, num_idxs, elem_size, elem_step,
                    sem=None):
    """bass.dma_gather without the elem_size_bytes%256 assert (non-transpose,
    DRAM source, 256B-multiple row stride). sem!=None -> prepare_only."""
    assert idxs_ap.dtype == mybir.dt.int16
    assert in_ap.dtype == out_ap.dtype
    assert ap_is_contiguous(out_ap.ap[1:])
    assert ap_is_contiguous(idxs_ap.ap[1:])
    assert in_ap.ap[0][0] == elem_step
    stride_bytes = elem_step * mybir.dt.size(in_ap.dtype)
    stride_bytes_256 = stride_bytes // 256
    assert stride_bytes_256 * 256 == stride_bytes and stride_bytes_256 < 256
    _in_ap = eng.lower_ap_dma(in_ap, for_custom_bir_dma=True)
    _idxs_ap = eng.lower_ap(idxs_ap)
    _out_ap = eng.lower_ap(out_ap)
    key = (id(eng.bass), num_idxs)
    if key not in _reg_cache:
        _reg_cache[key] = eng.to_reg(num_idxs)
    inst = eng.add_instruction(
        mybir.InstDMAGatherAnt(
            name=eng.bass.get_next_instruction_name(),
            ins=[*_in_ap, _idxs_ap, eng.lower_val_access(_reg_cache[key])],
            outs=[_out_ap],
            transpose=False,
            num_idxs=num_idxs,
            elem_size=elem_size,
            stride_bytes_256=stride_bytes_256,
            gen_mode=int(sem is not None),
            single_packet=False,
            queue_num=0,
            sbuf_tokens_per_rank=0,
            sbuf_free_dim_per_rank=0,
            sbuf_free_dim_pad_per_rank=0,
            sbuf_byte_offset=0,
        )
    )
    if sem is not None:
        inst.then_inc(sem, 16)
        return eng._track_prepare_only(inst, 0)
    return inst


def _wrap16(vals):
    """Edge-slot int16 index array -> dma_gather layout [128, n/16]."""
    n = len(vals)
    assert n % 16 == 0
    w = np.asarray(vals, np.int16).reshape(n // 16, 16).T
    return np.tile(w, (8, 1))


def _host_prep(x, edge_index, W, a_left, a_right):
    src = np.asarray(edge_index[0], np.int64)
    dst = np.asarray(edge_index[1], np.int64)

    # fold attention vectors through W:  [el|er] = x @ (W.T @ A)
    A = np.zeros((HC, 2 * H), np.float32)
    for h in range(H):
        A[h * C:(h + 1) * C, h] = a_left[h]
        A[h * C:(h + 1) * C, H + h] = a_right[h]
    B = (W.T.astype(np.float64) @ A.astype(np.float64)).astype(np.float32)
    wtb = np.zeros((IN, P1COLS), np.float32)
    for h in range(H):
        wtb[:, h * C1:h * C1 + C] = W.T[:, h * C:(h + 1) * C]
    wtb[:, WH1:WH1 + H] = B[:, :H]
    wtb[:, WH1 + H:] = B[:, H:]
    wtb = wtb.astype(BF16)

    core = dst // NPC
    r_src = src % NPC
    c_src = src // NPC
    is_a = r_src < ROWS_A
    gidx = np.where(is_a, c_src * ROWS_A + r_src,
                    c_src * ROWS_B + (r_src - ROWS_A)).astype(np.int64)

    capA = capB = 0
    lists = {}
    for c in range(NC):
        m = core == c
        s_c, d_c, g_c, a_c = src[m], dst[m], gidx[m], is_a[m]
        dl = d_c - c * NPC
        b_c = dl // BUCKET
        order = np.lexsort((s_c, b_c))
        s_c, dl, b_c, g_c, a_c = (s_c[order], dl[order], b_c[order],
                                  g_c[order], a_c[order])
        cnt = np.bincount(b_c, minlength=NBUCK)
        starts = np.concatenate([[0], np.cumsum(cnt)[:-1]])
        for b in range(NBUCK):
            sl = slice(starts[b], starts[b] + cnt[b])
            aa = a_c[sl]
            lists[(c, b)] = (g_c[sl][aa], dl[sl][aa] - b * BUCKET,
                             g_c[sl][~aa], dl[sl][~aa] - b * BUCKET)
            capA = max(capA, (int(aa.sum()) + 127) // 128)
            capB = max(capB, (int((~aa).sum()) + 127) // 128)

    nblkb = capA + capB
    nblk = NBUCK * nblkb
    nblkA = NBUCK * capA
    nblkB = NBUCK * capB
    n_sc = (NBUCK + SC_BUCKETS - 1) // SC_BUCKETS

    idxA = np.zeros((NC, 128, nblkA * 8), np.int16)
    idxB = np.zeros((NC, 128, nblkB * 8), np.int16)
    dloc_u = np.full((NC, 128, nblk), 200.0, BF16)
    dlocT = np.full((NC, 128, nblk * 128), -1, np.int8)
    xT = np.zeros((NC, IN, XT_PAD), BF16)

    for c in range(NC):
        iA = np.zeros(nblkA * 128, np.int64)
        iB = np.zeros(nblkB * 128, np.int64)
        dA = np.full((nblkA, 128), -1, np.int64)
        dB = np.full((nblkB, 128), -1, np.int64)
        for b in range(NBUCK):
            gA, dlA, gB, dlB = lists[(c, b)]
            oa = b * capA * 128
            ob = b * capB * 128
            iA[oa:oa + len(gA)] = gA
            iB[ob:ob + len(gB)] = gB
            fa = dA[b * capA:(b + 1) * capA].reshape(-1)
            fa[:len(dlA)] = dlA
            fb = dB[b * capB:(b + 1) * capB].reshape(-1)
            fb[:len(dlB)] = dlB
        idxA[c] = _wrap16(iA)
        idxB[c] = _wrap16(iB)
        # slot k of block j = (partition k%128); dA rows are flat slot runs
        dA = dA.reshape(nblkA, 128)
        dB = dB.reshape(nblkB, 128)
        # dloc_u: SC-major order [per SC: A-blocks | B-blocks], [128, nblk]
        off = 0
        for sc in range(n_sc):
            b0 = sc * SC_BUCKETS
            bs = range(b0, min(b0 + SC_BUCKETS, NBUCK))
            for b in bs:
                blk = dA[b * capA:(b + 1) * capA]     # [capA, 128]
                v = np.where(blk < 0, 200.0, blk).astype(np.float32)
                dloc_u[c, :, off:off + capA] = v.T.astype(BF16)
                off += capA
            for b in bs:
                blk = dB[b * capB:(b + 1) * capB]
                v = np.where(blk < 0, 200.0, blk).astype(np.float32)
                dloc_u[c, :, off:off + capB] = v.T.astype(BF16)
                off += capB
        # dlocT: BUCKET-major order [per bucket: A-blocks | B-blocks],
        # transposed and replicated: [128(any), (b*nblkb + j)*128 + p]
        for b in range(NBUCK):
            base = b * nblkb * 128
            rows = np.concatenate(
                [dA[b * capA:(b + 1) * capA], dB[b * capB:(b + 1) * capB]],
                axis=0)                                # [nblkb, 128]
            dlocT[c, :, base:base + nblkb * 128] = np.broadcast_to(
                rows.reshape(-1).astype(np.int8), (128, nblkb * 128))

        xs = x[c * NPC:(c + 1) * NPC].astype(BF16)
        xT[c, :, :NPC] = xs.T

    iota = np.tile(np.arange(128, dtype=np.float32)[None, :], (128, 1)).astype(BF16)
    iotaP = np.arange(128, dtype=np.int8).reshape(128, 1)
    iotaPb = np.arange(128, dtype=np.float32).reshape(128, 1).astype(BF16)
    return (wtb, idxA, idxB, dloc_u, dlocT, xT, iota, iotaP, iotaPb,
            capA, capB)


def _build_program(capA, capB):
    nblkb = capA + capB
    nblk = NBUCK * nblkb
    nblkA = NBUCK * capA
    nblkB = NBUCK * capB
    f32 = mybir.dt.float32
    bf16 = mybir.dt.bfloat16
    i16 = mybir.dt.int16
    i8 = mybir.dt.int8

    nc = bass.Bass(trn_type="TRN2", num_devices=NC)
    xT_in = nc.declare_dram_parameter("xT", [IN, XT_PAD], bf16, isOutput=False)
    wtb_in = nc.declare_dram_parameter("wtb", [IN, P1COLS], bf16, isOutput=False)
    idxA_in = nc.declare_dram_parameter("idxA", [128, nblkA * 8], i16, isOutput=False)
    idxB_in = nc.declare_dram_parameter("idxB", [128, nblkB * 8], i16, isOutput=False)
    dloc_in = nc.declare_dram_parameter("dloc", [128, nblk], bf16, isOutput=False)
    dlocT_in = nc.declare_dram_parameter("dlocT", [128, nblk * 128], i8, isOutput=False)
    iota_in = nc.declare_dram_parameter("iota", [128, 128], bf16, isOutput=False)
    iotaP_in = nc.declare_dram_parameter("iotaP", [128, 1], i8, isOutput=False)
    iotaPb_in = nc.declare_dram_parameter("iotaPb", [128, 1], bf16, isOutput=False)
    out_ext = nc.declare_dram_parameter("out", [NPC, HC], f32, isOutput=True)

    tbl_locA = nc.dram_tensor("tbl_locA", [ROWS_A, TROW], bf16)
    tbl_locB = nc.dram_tensor("tbl_locB", [ROWS_B, TROW], bf16)
    tblA = nc.dram_tensor("tblA", [NC * ROWS_A, TROW], bf16, addr_space="Shared")
    tblB = nc.dram_tensor("tblB", [NC * ROWS_B, TROW], bf16, addr_space="Shared")
    er_tbl = nc.dram_tensor("er_tbl", [NBUCK * 128, H], bf16)

    with tile.TileContext(nc) as tc:
        with tc.tile_pool(name="cst", bufs=1) as cst:
            # ---------------- phase 1: Wh1 / el / er ----------------
            with tc.tile_pool(name="p1w", bufs=1) as p1w, \
                 tc.tile_pool(name="p1", bufs=3) as p1, \
                 tc.tile_pool(name="ps1", bufs=2, space="PSUM") as ps1:
                xts = []
                wtbs = []
                for k in range(2):
                    t = p1w.tile([128, XT_PAD], bf16, tag=f"xt{k}")
                    nc.sync.dma_start(out=t[:], in_=xT_in[k * 128:(k + 1) * 128, :])
                    xts.append(t)
                    u = p1w.tile([128, P1COLS], bf16, tag=f"wtb{k}")
                    nc.sync.dma_start(out=u[:], in_=wtb_in[k * 128:(k + 1) * 128, :])
                    wtbs.append(u)
                for tn in range(NBUCK):
                    ps = ps1.tile([128, P1COLS], f32)
                    for k in range(2):
                        nc.tensor.matmul(
                            out=ps[:],
                            lhsT=xts[k][:, tn * 128:(tn + 1) * 128],
                            rhs=wtbs[k][:],
                            start=(k == 0), stop=(k == 1),
                        )
                    sb = p1.tile([128, P1COLS], bf16)
                    nc.scalar.activation(out=sb[:], in_=ps[:],
                                         func=mybir.ActivationFunctionType.Copy)
                    sb4 = sb[:, 0:WH1].rearrange("p (h c) -> p h c", c=C1)
                    nc.vector.memset(sb4[:, :, C:C1], 1.0)
                    rows = min(128, NPC - tn * 128)
                    if tn < NBUCK_A:
                        trow = tbl_locA[tn * 128:tn * 128 + rows, 0:PAY]
                    else:
                        r0 = tn * 128 - ROWS_A
                        trow = tbl_locB[r0:r0 + rows, 0:PAY]
                    nc.sync.dma_start(out=trow, in_=sb[:rows, 0:PAY])
                    nc.sync.dma_start(
                        out=er_tbl[tn * 128:tn * 128 + rows, :],
                        in_=sb[:rows, PAY:P1COLS])

            # ---------------- all-gather both half tables ----------------
            nc.gpsimd.collective_compute(
                "AllGather", mybir.AluOpType.bypass,
                replica_groups=[list(range(NC))],
                ins=[tbl_locA[:].opt()], outs=[tblA[:].opt()],
            )
            nc.gpsimd.collective_compute(
                "AllGather", mybir.AluOpType.bypass,
                replica_groups=[list(range(NC))],
                ins=[tbl_locB[:].opt()], outs=[tblB[:].opt()],
            )
            nc.gpsimd.load_library(library_config.mlp)

            iota_b = cst.tile([128, 128], bf16)
            nc.sync.dma_start(out=iota_b[:], in_=iota_in[:, :])
            iota_m = iota_b[:].rearrange("p (b n) -> p b n", b=1)
            iotaP_sb = cst.tile([128, 1], i8)
            nc.sync.dma_start(out=iotaP_sb[:], in_=iotaP_in[:, :])
            iotaP_m = iotaP_sb[:].rearrange("p (b n) -> p b n", b=1)
            iotaPb_sb = cst.tile([128, 1], bf16)
            nc.sync.dma_start(out=iotaPb_sb[:], in_=iotaPb_in[:, :])
            ident = cst.tile([128, 128], bf16)
            nc.vector.tensor_tensor(
                out=ident[:], in0=iotaPb_sb[:].to_broadcast([128, 128]),
                in1=iota_b[:], op=mybir.AluOpType.is_equal)

            idxA_sb = cst.tile([128, nblkA * 8], i16)
            nc.sync.dma_start(out=idxA_sb[:], in_=idxA_in[:, :])
            idxB_sb = cst.tile([128, nblkB * 8], i16)
            nc.sync.dma_start(out=idxB_sb[:], in_=idxB_in[:, :])
            dloc_sb = cst.tile([128, nblk], bf16)
            nc.sync.dma_start(out=dloc_sb[:], in_=dloc_in[:, :])
            # er for all buckets, bucket-partition layout: [p, b, h]
            er_full = cst.tile([128, NBUCK * H], bf16)
            er_full3 = er_full[:].rearrange("p (b h) -> p b h", h=H)
            er_in3 = er_tbl[:(NBUCK - 1) * 128].rearrange("(b p) h -> p b h", p=128)
            nc.sync.dma_start(out=er_full3[:, 0:NBUCK - 1, :], in_=er_in3[:, :, :])
            # last bucket is partial (106 rows): zero first so the OTT matmul
            # can't pick up non-finite garbage from the dead partitions
            nc.vector.memset(er_full3[:, NBUCK - 1, :], 0.0)
            lastr = NPC - (NBUCK - 1) * 128
            nc.sync.dma_start(
                out=er_full3[0:lastr, NBUCK - 1, :],
                in_=er_tbl[(NBUCK - 1) * 128:NPC, :])

            # ---------------- er expansion on PE (overlaps AllGather) ------
            er_e = cst.tile([128, nblk * 8], bf16)   # SC-major slot order
            with tc.tile_pool(name="otq", bufs=2) as otq, \
                 tc.tile_pool(name="dtq", bufs=2) as dtq, \
                 tc.tile_pool(name="pse", bufs=2, space="PSUM") as pse:
                n_sc = (NBUCK + SC_BUCKETS - 1) // SC_BUCKETS
                for b in range(NBUCK):
                    dT = dtq.tile([128, nblkb * 128], i8, tag="dT")
                    nc.sync.dma_start(
                        out=dT[:],
                        in_=dlocT_in[:, b * nblkb * 128:(b + 1) * nblkb * 128])
                    dT3 = dT[:].rearrange("p (j n) -> p j n", n=128)
                    OTT = otq.tile([128, nblkb * 128], bf16, tag="OTT")
                    OTT3 = OTT[:].rearrange("p (j n) -> p j n", n=128)
                    nc.vector.tensor_tensor(
                        out=OTT3, in0=iotaP_m.to_broadcast([128, nblkb, 128]),
                        in1=dT3, op=mybir.AluOpType.is_equal)
                    pe = pse.tile([128, nblkb * H], f32)
                    pe3 = pe[:].rearrange("p (j h) -> p j h", h=H)
                    for j in range(nblkb):
                        nc.tensor.matmul(
                            out=pe3[:, j, :], lhsT=OTT3[:, j, :],
                            rhs=er_full3[:, b, :], start=True, stop=True)
                    # copy into er_e at SC-major positions (A-run | B-run)
                    sc = b // SC_BUCKETS
                    b0 = sc * SC_BUCKETS
                    nbk = min(SC_BUCKETS, NBUCK - b0)
                    off_sc = b0 * nblkb
                    offA = (off_sc + (b - b0) * capA) * 8
                    offB = (off_sc + nbk * capA + (b - b0) * capB) * 8
                    nc.vector.tensor_copy(
                        out=er_e[:, offA:offA + capA * 8],
                        in_=pe[:, 0:capA * 8])
                    nc.vector.tensor_copy(
                        out=er_e[:, offB:offB + capB * 8],
                        in_=pe[:, capA * 8:nblkb * 8])

            # ---------------- phase 2: gather / score / scatter ------------
            with tc.tile_pool(name="gp", bufs=2) as gp, \
                 tc.tile_pool(name="vp", bufs=2) as vp, \
                 tc.tile_pool(name="otp", bufs=2) as otp, \
                 tc.tile_pool(name="sp", bufs=2) as sp, \
                 tc.tile_pool(name="sf", bufs=3) as sf, \
                 tc.tile_pool(name="np_", bufs=3) as np_, \
                 tc.tile_pool(name="ps2", bufs=4, space="PSUM") as ps2p:

                gsem = nc.alloc_semaphore("gsem") if USE_PREP_TRIGGER else None
                n_sc = (NBUCK + SC_BUCKETS - 1) // SC_BUCKETS
                off_u = 0
                for sc in range(n_sc):
                    b0 = sc * SC_BUCKETS
                    nbk = min(SC_BUCKETS, NBUCK - b0)
                    nbA = nbk * capA
                    nbB = nbk * capB
                    nb = nbA + nbB

                    G = gp.tile([128, nb * GELEM], bf16, tag="G")
                    G3 = G[:].rearrange("p (b y) -> p b y", y=GELEM)
                    a0 = b0 * capA * 8
                    bb0 = b0 * capB * 8
                    if USE_PREP_TRIGGER:
                        _dma_gather_raw(
                            nc.gpsimd, G3[:, 0:nbA, :], tblA[:],
                            idxA_sb[:, a0:a0 + nbA * 8], nbA * 128,
                            GELEM, TROW, sem=gsem)
                        nc.gpsimd.trigger_dma(count=None)
                        _dma_gather_raw(
                            nc.gpsimd, G3[:, nbA:nb, :], tblB[:],
                            idxB_sb[:, bb0:bb0 + nbB * 8], nbB * 128,
                            GELEM, TROW, sem=gsem)
                        nc.gpsimd.trigger_dma(count=None)
                    else:
                        _dma_gather_raw(
                            nc.gpsimd, G3[:, 0:nbA, :], tblA[:],
                            idxA_sb[:, a0:a0 + nbA * 8], nbA * 128,
                            GELEM, TROW)
                        _dma_gather_raw(
                            nc.gpsimd, G3[:, nbA:nb, :], tblB[:],
                            idxB_sb[:, bb0:bb0 + nbB * 8], nbB * 128,
                            GELEM, TROW)

                    # scores: e = el + er ; leaky ; exp
                    e_t = sp.tile([128, nb * H], f32, tag="e")
                    e3 = e_t[:].rearrange("p (b h) -> p b h", h=H)
                    er_sc3 = er_e[:, off_u * 8:(off_u + nb) * 8].rearrange(
                        "p (b h) -> p b h", h=H)
                    nc.vector.tensor_tensor(
                        out=e3, in0=G3[:, :, WH1:PAY], in1=er_sc3,
                        op=mybir.AluOpType.add)
                    # w = exp(leaky(e)) = max(exp(e), exp(NEG*e)) (exp monotone)
                    w_t = sp.tile([128, nb * H], bf16, tag="w")
                    w1_t = sp.tile([128, nb * H], bf16, tag="w1")
                    nc.scalar.activation(
                        out=w1_t[:], in_=e_t[:],
                        func=mybir.ActivationFunctionType.Exp)
                    w2_t = sp.tile([128, nb * H], bf16, tag="w2")
                    nc.scalar.activation(
                        out=w2_t[:], in_=e_t[:], scale=NEG,
                        func=mybir.ActivationFunctionType.Exp)
                    nc.vector.tensor_tensor(
                        out=w_t[:], in0=w1_t[:], in1=w2_t[:],
                        op=mybir.AluOpType.max)
                    w3 = w_t[:].rearrange("p (b h) -> p b h", h=H)

                    # one-op one-hot build for the whole super-chunk
                    OT = otp.tile([128, nb * 128], bf16, tag="OT")
                    OT3 = OT[:].rearrange("p (b n) -> p b n", n=128)
                    d3 = dloc_sb[:, off_u:off_u + nb].to_broadcast([128, nb, 128])
                    i3 = iota_m.to_broadcast([128, nb, 128])
                    nc.vector.tensor_tensor(out=OT3, in0=d3, in1=i3,
                                            op=mybir.AluOpType.is_equal)

                    # per-block V multiplies
                    V = vp.tile([128, nb * WH1], bf16, tag="V")
                    V3 = V[:].rearrange("p (b y) -> p b y", y=WH1)
                    for blk in range(nb):
                        V4b = V3[:, blk, :].rearrange("p (h c) -> p h c", c=C1)
                        G4b = G3[:, blk, 0:WH1].rearrange("p (h c) -> p h c", c=C1)
                        w4b = w3[:, blk, :].to_broadcast([128, H, C1])
                        nc.vector.tensor_tensor(out=V4b, in0=G4b, in1=w4b,
                                                op=mybir.AluOpType.mult)

                    # per bucket: scatter matmuls + self-loop fold + normalize
                    for bb in range(nbk):
                        bucket = b0 + bb
                        # self-loop contribution
                        ts = sf.tile([128, PAY], bf16, tag="ts")
                        if bucket < NBUCK_A:
                            tsrc = tbl_locA[bucket * 128:bucket * 128 + 128, 0:PAY]
                        else:
                            r0 = bucket * 128 - ROWS_A
                            rows = min(128, ROWS_B - r0)
                            tsrc = tbl_locB[r0:r0 + rows, 0:PAY]
                        nc.sync.dma_start(out=ts[:tsrc.shape[0], :], in_=tsrc)
                        es_s = sf.tile([128, H], f32, tag="es_s")
                        nc.vector.tensor_tensor(
                            out=es_s[:], in0=ts[:, WH1:PAY],
                            in1=er_full3[:, bucket, :], op=mybir.AluOpType.add)
                        ws_s = sf.tile([128, H], bf16, tag="ws_s")
                        ws1 = sf.tile([128, H], bf16, tag="ws1")
                        nc.scalar.activation(
                            out=ws1[:], in_=es_s[:],
                            func=mybir.ActivationFunctionType.Exp)
                        ws2 = sf.tile([128, H], bf16, tag="ws2")
                        nc.scalar.activation(
                            out=ws2[:], in_=es_s[:], scale=NEG,
                            func=mybir.ActivationFunctionType.Exp)
                        nc.vector.tensor_tensor(
                            out=ws_s[:], in0=ws1[:], in1=ws2[:],
                            op=mybir.AluOpType.max)
                        vs = sf.tile([128, WH1], bf16, tag="vs")
                        vs4 = vs[:].rearrange("p (h c) -> p h c", c=C1)
                        ts4 = ts[:, 0:WH1].rearrange("p (h c) -> p h c", c=C1)
                        nc.vector.tensor_tensor(
                            out=vs4, in0=ts4,
                            in1=ws_s[:].to_broadcast([128, H, C1]),
                            op=mybir.AluOpType.mult)

                        blks = ([bb * capA + j for j in range(capA)] +
                                [nbA + bb * capB + j for j in range(capB)])
                        ps = ps2p.tile([128, WH1], f32)
                        for i, blk in enumerate(blks):
                            nc.tensor.matmul(
                                out=ps[:], lhsT=OT3[:, blk, :], rhs=V3[:, blk, :],
                                start=(i == 0), stop=False)
                        nc.tensor.matmul(
                            out=ps[:], lhsT=ident[:], rhs=vs[:],
                            start=False, stop=True)

                        ps4 = ps[:].rearrange("p (h c) -> p h c", c=C1)
                        den = np_.tile([128, H], f32, tag="den")
                        nc.vector.tensor_scalar_add(den[:], ps4[:, :, C], EPS)
                        rec = np_.tile([128, H], f32, tag="rec")
                        nc.vector.reciprocal(rec[:], den[:])
                        ot = np_.tile([128, HC], f32, tag="ot")
                        ot3 = ot[:].rearrange("p (h c) -> p h c", c=C)
                        r3 = rec[:].to_broadcast([128, H, C])
                        nc.vector.tensor_tensor(out=ot3, in0=ps4[:, :, 0:C],
                                                in1=r3, op=mybir.AluOpType.mult)
                        rows = min(128, NPC - bucket * 128)
                        nc.sync.dma_start(
                            out=out_ext[bucket * 128:bucket * 128 + rows, :],
                            in_=ot[:rows, :])
                    off_u += nb

    _split_excess_waits(nc)
    _move_reload_after_collectives(nc)
    lower_extended_insts(nc)
    return nc


def kernel(**inputs):
    x = np.asarray(inputs["x"], np.float32)
    edge_index = np.asarray(inputs["edge_index"])
    W = np.asarray(inputs["W"], np.float32)
    a_left = np.asarray(inputs["a_left"], np.float32)
    a_right = np.asarray(inputs["a_right"], np.float32)

    (wtb, idxA, idxB, dloc_u, dlocT, xT, iota, iotaP, iotaPb,
     capA, capB) = _host_prep(x, edge_index, W, a_left, a_right)
    nc = _build_program(capA, capB)

    in_maps = []
    for c in range(NC):
        in_maps.append({
            "xT": np.ascontiguousarray(xT[c]),
            "wtb": wtb,
            "idxA": np.ascontiguousarray(idxA[c]),
            "idxB": np.ascontiguousarray(idxB[c]),
            "dloc": np.ascontiguousarray(dloc_u[c]),
            "dlocT": np.ascontiguousarray(dlocT[c]),
            "iota": iota,
            "iotaP": iotaP,
            "iotaPb": iotaPb,
        })

    res = run_bass_kernel_spmd(nc, in_maps, core_ids=list(range(NC)))
    out = np.concatenate([np.asarray(res.results[c]["out"]) for c in range(NC)], axis=0)
    return out.astype(np.float32)
